# revision 2
# baseline (speedup 1.0000x reference)
"""BigBird sparse attention on 8 Trainium2 NeuronCores — v2.

Sharding: 16 heads across 8 cores (2 heads/core, both batches per core).
Per core: q/k/v projections for its 2 heads, block-sparse BigBird attention,
partial output projection (contracting both heads at once, K=128), output
written as outT [D, R] bf16; host sums the 8 partials (transposed) + o_b.

v2 changes vs baseline:
- Random-key K/V fetched with ROW-mode dma_gather only (no transpose-mode
  SWDGE gathers); K chunks transposed to column layout on the PE.
- V gathered with a denominator-rider column (1.0) per (b, h) so every PV
  matmul is a single M=65 accumulation (no separate denominator matmuls).
- QK/PV restructured j-major: global key block j=0 processed with N=512
  matmuls over all queries; band key blocks j>=1 with N<=384 matmuls over
  the 3 adjacent query blocks; PV accumulates into one [65, S] PSUM region
  per (b, h) using partial-region start/stop groups.
- Normalization via tensor_tensor divide against a PE-broadcast denominator.
- Output projection contracts both heads per matmul (K=128) into oT layout.
"""

import math
import numpy as np

# ---------------------------------------------------------------- constants
B = 2
S = 2048
D = 1024
H = 16
HD = 64
NUM_GLOBAL = 2
NUM_RANDOM = 3
WINDOW = 3

N_CORES = 8
HPC = H // N_CORES          # heads per core = 2
HD2 = HPC * HD              # 128 = head-dim slice per core
R = B * S                   # 4096 flattened rows
NT = S // 128               # 16 i-blocks / j-blocks
NRG = NUM_RANDOM * 128      # gathered random keys per i-block = 384
NIDX = NT * NRG             # 6144 gather indices (each row holds both b)
NCH = NIDX // 128           # 48 gathered chunks of 128 keys

INV_SQRT_HD = 1.0 / math.sqrt(float(HD))

# j-major band spans: i-block range [TLO[j], THI[j]] uses key block j as a
# band chunk. j=0 is the global pass (all i-blocks).
TLO = [0] + [max(j - 1, 0) for j in range(1, NT)]
THI = [NT - 1] + [min(j + 1, NT - 1) for j in range(1, NT)]
BSPAN = [128 * (THI[j] - TLO[j] + 1) for j in range(NT)]  # cols; j=0: 2048

# mask layout: [glob 2048][band j=1..15][rand 48*128]
MOFF_BAND = [0] * NT
off = BSPAN[0]
for j in range(1, NT):
    MOFF_BAND[j] = off
    off += BSPAN[j]
MOFF_RAND = off
NMASK = off + NCH * 128


# ---------------------------------------------------------------- host prep
def _build_ref_mask(random_indices):
    i = np.arange(S)[:, None]
    j = np.arange(S)[None, :]
    glob = (i < NUM_GLOBAL) | (j < NUM_GLOBAL)
    win = np.abs(i - j) <= WINDOW
    rand = np.zeros((S, S), dtype=bool)
    rows = np.repeat(np.arange(S), NUM_RANDOM)
    rand[rows, random_indices.reshape(-1)] = True
    return glob | win | rand


def _host_masks_and_idx(random_indices):
    """j-major band masks + rand chunk masks [128, NMASK] bf16, gather idx."""
    import ml_dtypes

    ri = np.asarray(random_indices).astype(np.int64)
    M = _build_ref_mask(ri)

    masks = np.zeros((128, NMASK), dtype=np.float32)
    # global + band: exact reference mask restricted to (j-block, i-span),
    # transposed to [j-key, i].
    for j in range(NT):
        moff = MOFF_BAND[j]
        ilo, ihi = TLO[j] * 128, (THI[j] + 1) * 128
        sub = M[ilo:ihi, j * 128:(j + 1) * 128]  # [i, j]
        masks[:, moff:moff + (ihi - ilo)] = sub.T.astype(np.float32)
    # rand chunks: c = 3t + g covers key n = 128g + p of i-block t
    for t in range(NT):
        covered = {jb for jb in (t - 1, t, t + 1) if 0 <= jb < NT} | {0}
        for g in range(NUM_RANDOM):
            c = NUM_RANDOM * t + g
            blk = np.zeros((128, 128), dtype=np.float32)
            for p in range(128):
                n = 128 * g + p
                il = n // NUM_RANDOM
                m = n % NUM_RANDOM
                i_glob = t * 128 + il
                r_ = ri[i_glob, m]
                if r_ // 128 in covered:
                    continue
                if any(ri[i_glob, mm] == r_ for mm in range(m)):
                    continue
                blk[p, il] = 1.0
            masks[:, MOFF_RAND + c * 128:MOFF_RAND + (c + 1) * 128] = blk
    masks_bf16 = masks.astype(ml_dtypes.bfloat16)

    # gather indices: flat order n = t*384 + 128*g + p
    n = np.arange(NIDX)
    t_of = n // NRG
    g_of = (n % NRG) // 128
    p_of = n % 128
    nn = 128 * g_of + p_of
    il = nn // NUM_RANDOM
    m = nn % NUM_RANDOM
    j_of = ri[t_of * 128 + il, m]
    vals = j_of.astype(np.int16)
    a16 = np.zeros((16, NIDX // 16), dtype=np.int16)
    a16[n % 16, n // 16] = vals
    return masks_bf16, np.tile(a16, (8, 1))


def make_in_maps(inputs):
    import ml_dtypes

    x = np.asarray(inputs["x"], dtype=np.float32)
    ri = np.asarray(inputs["random_indices"])
    q_w = np.asarray(inputs["q_w"], dtype=np.float32)
    k_w = np.asarray(inputs["k_w"], dtype=np.float32)
    v_w = np.asarray(inputs["v_w"], dtype=np.float32)
    o_w = np.asarray(inputs["o_w"], dtype=np.float32)
    q_b = np.asarray(inputs["q_b"], dtype=np.float32)
    k_b = np.asarray(inputs["k_b"], dtype=np.float32)
    v_b = np.asarray(inputs["v_b"], dtype=np.float32)

    xT = np.ascontiguousarray(x.reshape(R, D).T).astype(ml_dtypes.bfloat16)
    masks, gidx = _host_masks_and_idx(ri)

    q_w = q_w * INV_SQRT_HD
    q_b = q_b * INV_SQRT_HD

    in_maps = []
    for c in range(N_CORES):
        sl = slice(HD2 * c, HD2 * (c + 1))
        in_maps.append({
            "xT": xT,
            "wq": np.ascontiguousarray(q_w[sl, :].T).astype(ml_dtypes.bfloat16),
            "wk": np.ascontiguousarray(k_w[sl, :].T).astype(ml_dtypes.bfloat16),
            "wv": np.ascontiguousarray(v_w[sl, :].T).astype(ml_dtypes.bfloat16),
            "bq": np.ascontiguousarray(q_b[sl, None]),
            "bk": np.ascontiguousarray(k_b[sl, None]),
            "bv": np.ascontiguousarray(v_b[sl, None]),
            "wo": np.ascontiguousarray(o_w[:, sl].T).astype(ml_dtypes.bfloat16),
            "masks": masks,
            "gidx": gidx,
        })
    return in_maps


# ---------------------------------------------------------------- device IR
def build_kernel():
    import concourse.tile as tile
    from concourse import bacc, mybir

    nc = bacc.Bacc("TRN2", target_bir_lowering=False, debug=False,
                   num_swdge_queues=4)
    f32 = mybir.dt.float32
    bf16 = mybir.dt.bfloat16
    i16 = mybir.dt.int16

    t_ = dict(
        xT=nc.dram_tensor("xT", [D, R], bf16, kind="ExternalInput").ap(),
        wq=nc.dram_tensor("wq", [D, HD2], bf16, kind="ExternalInput").ap(),
        wk=nc.dram_tensor("wk", [D, HD2], bf16, kind="ExternalInput").ap(),
        wv=nc.dram_tensor("wv", [D, HD2], bf16, kind="ExternalInput").ap(),
        bq=nc.dram_tensor("bq", [HD2, 1], f32, kind="ExternalInput").ap(),
        bk=nc.dram_tensor("bk", [HD2, 1], f32, kind="ExternalInput").ap(),
        bv=nc.dram_tensor("bv", [HD2, 1], f32, kind="ExternalInput").ap(),
        wo=nc.dram_tensor("wo", [HD2, D], bf16, kind="ExternalInput").ap(),
        masks=nc.dram_tensor("masks", [128, NMASK], bf16,
                             kind="ExternalInput").ap(),
        gidx=nc.dram_tensor("gidx", [128, NIDX // 16], i16,
                            kind="ExternalInput").ap(),
        outT=nc.dram_tensor("outT", [D, R], bf16, kind="ExternalOutput").ap(),
        v_stage=nc.dram_tensor("v_stage", [S, B, HPC, 128], bf16).ap(),
    )

    with tile.TileContext(nc) as tc:
        _build_tc(nc, tc, t_)
    nc.compile()
    return nc


def _build_tc(nc, tc, t_):
    import os
    from contextlib import ExitStack
    STAGES = os.environ.get("K_STAGES", "ABGCD")

    import concourse.bass as bass
    from concourse import masks as cmasks, mybir

    f32 = mybir.dt.float32
    bf16 = mybir.dt.bfloat16
    EXP = mybir.ActivationFunctionType.Exp
    MULT = mybir.AluOpType.mult
    DIV = mybir.AluOpType.divide

    with ExitStack() as ctx:
        const = ctx.enter_context(tc.tile_pool(name="const", bufs=1))
        proj = ctx.enter_context(tc.tile_pool(name="proj", bufs=1))
        persist = ctx.enter_context(tc.tile_pool(name="persist", bufs=1))

        # ---- constants
        ident = const.tile([128, 128], bf16)
        cmasks.make_identity(nc, ident[:])
        ones_f32 = const.tile([128, HD], f32)
        nc.vector.memset(ones_f32[:], 1.0)

        wq_sb = const.tile([128, 8, HD2], bf16)
        wk_sb = const.tile([128, 8, HD2], bf16)
        wv_sb = const.tile([128, 8, HD2], bf16)
        for w_sb, w_d in ((wq_sb, t_["wq"]), (wk_sb, t_["wk"]),
                          (wv_sb, t_["wv"])):
            nc.sync.dma_start(w_sb[:], w_d.rearrange("(c p) m -> p c m", p=128))
        bq_sb = const.tile([HD2, 1], f32)
        bk_sb = const.tile([HD2, 1], f32)
        bv_sb = const.tile([HD2, 1], f32)
        nc.sync.dma_start(bq_sb[:], t_["bq"])
        nc.sync.dma_start(bk_sb[:], t_["bk"])
        nc.sync.dma_start(bv_sb[:], t_["bv"])
        wo_sb = const.tile([HD2, D], bf16)
        nc.sync.dma_start(wo_sb[:], t_["wo"])
        mask_sb = const.tile([128, NMASK], bf16)
        nc.sync.dma_start(mask_sb[:], t_["masks"])
        gidx_sb = const.tile([128, NIDX // 16], mybir.dt.int16)
        nc.sync.dma_start(gidx_sb[:], t_["gidx"])

        # persistent activations
        qT = proj.tile([128, R], bf16)
        kT = proj.tile([128, R], bf16)
        # both heads' attention outputs stacked: h0 -> partitions 0..63
        houtb = proj.tile([128, B, S], bf16)
        # v rows + ones rider at col HD, per (b, h)
        vaug = [[persist.tile([128, NT, 128], bf16, name=f"vaug{b}{h}")
                 for h in range(HPC)] for b in range(B)]
        for b in range(B):
            for h in range(HPC):
                nc.vector.memset(vaug[b][h][:, :, HD + 1:], 0.0)
                nc.vector.memset(vaug[b][h][:, :, HD:HD + 1], 1.0)
        # gathered tensors; vselaug free layout per chunk:
        # [b0h0 d0..63, one, pad*63, b0h1 ..., b1h0 ..., b1h1 ...]
        # kselT/vselaug are whole-tile copies made after ALL gathers land —
        # the copy's whole-tile read waits on final (order-independent) DMASW
        # sem counts, making the 4-queue gather spread race-free. vselaug is
        # compacted to the 65 used cols per (b, h).
        vselaug = persist.tile([128, NCH, B * HPC, HD + 1], bf16)
        kselT = persist.tile([128, B, NIDX // 768, 768], bf16)
        krows_p = [persist.tile([128, NT, HD2], bf16, name=f"krowsp{b}")
                   for b in range(B)]

        # ---- stage A: projections -> qT/kT/vT [128(hd2), R] bf16
        with tc.tile_pool(name="stgA", bufs=1) as stga, \
                tc.tile_pool(name="xstream", bufs=2) as xpool, \
                tc.tile_pool(name="ppsum", bufs=3, space="PSUM") as ppsum:
            vT = stga.tile([128, R], bf16)
            xT_r = t_["xT"].rearrange("(c p) r -> p c r", p=128)
            for rt in range(R // 512):
                xt = xpool.tile([128, 8, 512], bf16)
                nc.sync.dma_start(xt[:], xT_r[:, :, bass.ts(rt, 512)])
                for dst, w_sb, b_sb in ((qT, wq_sb, bq_sb), (kT, wk_sb, bk_sb),
                                        (vT, wv_sb, bv_sb)):
                    ps = ppsum.tile([128, 512], f32, tag="ps")
                    for dc in range(8):
                        nc.tensor.matmul(ps[:], w_sb[:, dc, :], xt[:, dc, :],
                                         start=(dc == 0), stop=(dc == 7))
                    nc.vector.tensor_scalar_add(dst[:, bass.ts(rt, 512)],
                                                ps[:], b_sb[:])

            # ---- stage B: k/v row-major staging (PE transposes)
            with tc.tile_pool(name="stgB", bufs=1) as stg, \
                    tc.tile_pool(name="tpsum", bufs=4, space="PSUM") as tpsum:
                for b in range(B):
                    krows = krows_p[b]
                    for jc in range(NT):
                        csl = slice(b * S + jc * 128, b * S + (jc + 1) * 128)
                        psk = tpsum.tile([128, 128], bf16, tag="tp")
                        nc.tensor.transpose(psk[:], kT[:, csl], ident[:])
                        nc.vector.tensor_copy(krows[:, jc, :], psk[:])
                        psv = tpsum.tile([128, 128], bf16, tag="tp")
                        nc.tensor.transpose(psv[:], vT[:, csl], ident[:])
                        nc.vector.tensor_copy(vaug[b][0][:, jc, 0:HD],
                                              psv[:, 0:HD])
                        nc.vector.tensor_copy(vaug[b][1][:, jc, 0:HD],
                                              psv[:, HD:HD2])
                    for h in range(HPC):
                        nc.sync.dma_start(
                            t_["v_stage"][:, b, h, :].rearrange(
                                "(c p) e -> p c e", p=128),
                            vaug[b][h][:, :, :])

        # ---- gathers: V rows from DRAM; K via SBUF-source transpose
        # gathers from krows (column-layout output, no PE transposes).
        # 768 idx per call (49 SWDGE FIFO entries), spread over 4 queues.
        CR = 768
        NCALL = NIDX // CR  # 8
        v_src = t_["v_stage"].rearrange("j b h e -> j (b h e)")
        with tc.tile_pool(name="graw", bufs=1) as graw:
            kselT0 = graw.tile([128, B, NIDX // 768, 768], bf16)
            vselaug0 = graw.tile([128, NCH, B * HPC * 128], bf16)
            # queue = emission_index % 4 keeps each DMASW sem lane (index % 8)
            # on a single queue, preserving per-lane completion order.
            gq = [0]

            def nextq():
                q = gq[0] % 4
                gq[0] += 1
                return q

            for u in range(NCALL):
                isl = slice(u * (CR // 16), (u + 1) * (CR // 16))
                if "G" in STAGES:
                    for b in range(B):
                        nc.gpsimd.dma_gather(
                            kselT0[:, b, u:u + 1, :], krows_p[b][:],
                            gidx_sb[:, isl], CR, CR, HD2, transpose=True,
                            queue_num=nextq(),
                            sbuf_tokens_per_rank=128,
                            sbuf_free_dim_per_rank=HD2 * 2)
                if "V" not in STAGES:
                    nc.gpsimd.dma_gather(
                        vselaug0[:, u * (CR // 128):(u + 1) * (CR // 128), :],
                        v_src, gidx_sb[:, isl], CR, CR, B * HPC * 128,
                        transpose=False, queue_num=nextq())
            nc.vector.tensor_copy(kselT[:], kselT0[:])
            v0v = vselaug0[:].rearrange("p c (x e) -> p c x e", e=128)
            nc.scalar.copy(vselaug[:], v0v[:, :, :, 0:HD + 1])

            if "C" not in STAGES:
                return
            # ---- stage C: attention per (b, h)
            with tc.tile_pool(name="pglob", bufs=2) as pgpool, \
                    tc.tile_pool(name="pband", bufs=4) as pbpool, \
                    tc.tile_pool(name="norm", bufs=4) as norm, \
                    tc.tile_pool(name="spsum", bufs=2, space="PSUM") as spsum, \
                    tc.tile_pool(name="vpsum", bufs=1, space="PSUM") as vpsum:
                for b in range(B):
                    for h in range(HPC):
                        hs = slice(HD * h, HD * (h + 1))
                        base = b * S
                        pvo = vpsum.tile([HD + 1, S], f32, tag="pv")

                        # work items: QK(j) producing p, then PV(j) consuming
                        # it one step behind, to keep the PE stream dense.
                        p_glob = pgpool.tile([128, S], bf16, tag="pg")
                        p_band = {}
                        p_rand = {}

                        def qk_glob():
                            for w in range(4):
                                ssc = spsum.tile([128, 512], f32, tag="s")
                                nc.tensor.matmul(
                                    ssc[:], kT[hs, base:base + 128],
                                    qT[hs, base + w * 512:base + (w + 1) * 512],
                                    start=True, stop=True)
                                nc.scalar.activation(
                                    p_glob[:, w * 512:(w + 1) * 512], ssc[:],
                                    EXP)
                            nc.vector.tensor_tensor(
                                out=p_glob[:], in0=p_glob[:],
                                in1=mask_sb[:, 0:S], op=MULT)

                        def qk_band(j):
                            span = BSPAN[j]
                            ilo = TLO[j] * 128
                            ssc = spsum.tile([128, 512], f32, tag="s")
                            nc.tensor.matmul(
                                ssc[:, 0:span],
                                kT[hs, base + j * 128:base + (j + 1) * 128],
                                qT[hs, base + ilo:base + ilo + span],
                                start=True, stop=True)
                            pb = pbpool.tile([128, 384], bf16, tag="pb",
                                             name=f"pb{b}{h}_{j}")
                            p_band[j] = pb
                            nc.scalar.activation(pb[:, 0:span], ssc[:, 0:span],
                                                 EXP)
                            moff = MOFF_BAND[j]
                            nc.vector.tensor_tensor(
                                out=pb[:, 0:span], in0=pb[:, 0:span],
                                in1=mask_sb[:, moff:moff + span], op=MULT)

                        def qk_rand(t):
                            ssc = spsum.tile([128, 512], f32, tag="s")
                            for g in range(NUM_RANDOM):
                                n0 = t * NRG + g * 128
                                nc.tensor.matmul(
                                    ssc[:, g * 128:(g + 1) * 128],
                                    kselT[hs, b, n0 // 768,
                                          n0 % 768:n0 % 768 + 128],
                                    qT[hs, base + t * 128:base + (t + 1) * 128],
                                    start=True, stop=True)
                            pr = pbpool.tile([128, 384], bf16, tag="pr",
                                             name=f"pr{b}{h}_{t}")
                            p_rand[t] = pr
                            nc.scalar.activation(pr[:], ssc[:, 0:NRG], EXP)
                            m0 = MOFF_RAND + t * NRG
                            nc.vector.tensor_tensor(
                                out=pr[:], in0=pr[:],
                                in1=mask_sb[:, m0:m0 + NRG], op=MULT)

                        def pv_glob():
                            # init whole [65, S] region (global keys attend
                            # to every query)
                            for w in range(4):
                                nc.tensor.matmul(
                                    pvo[:, w * 512:(w + 1) * 512],
                                    vaug[b][h][:, 0, 0:HD + 1],
                                    p_glob[:, w * 512:(w + 1) * 512],
                                    start=True, stop=False,
                                    skip_group_check=True)

                        def pv_band(j):
                            span = BSPAN[j]
                            ilo = TLO[j] * 128
                            # split at PSUM bank (512-col) boundaries
                            lo = ilo
                            while lo < ilo + span:
                                hi = min(ilo + span, (lo // 512 + 1) * 512)
                                nc.tensor.matmul(
                                    pvo[:, lo:hi],
                                    vaug[b][h][:, j, 0:HD + 1],
                                    p_band[j][:, lo - ilo:hi - ilo],
                                    start=False, stop=False,
                                    skip_group_check=True)
                                lo = hi
                            del p_band[j]

                        def pv_rand(t):
                            for g in range(NUM_RANDOM):
                                c = NUM_RANDOM * t + g
                                nc.tensor.matmul(
                                    pvo[:, t * 128:(t + 1) * 128],
                                    vselaug[:, c, b * HPC + h, :],
                                    p_rand[t][:, g * 128:(g + 1) * 128],
                                    start=False, stop=(g == NUM_RANDOM - 1),
                                    skip_group_check=True)
                            del p_rand[t]

                        # software-pipelined emission: QK one step ahead of PV
                        work_qk = ([qk_glob]
                                   + [lambda j=j: qk_band(j)
                                      for j in range(1, NT)]
                                   + [lambda t=t: qk_rand(t)
                                      for t in range(NT)])
                        work_pv = ([pv_glob]
                                   + [lambda j=j: pv_band(j)
                                      for j in range(1, NT)]
                                   + [lambda t=t: pv_rand(t)
                                      for t in range(NT)])
                        work_qk[0]()
                        for i in range(len(work_pv)):
                            if i + 1 < len(work_qk):
                                work_qk[i + 1]()
                            work_pv[i]()

                        # normalize: hout = pv / den per 512-col group
                        for w in range(4):
                            wsl = slice(w * 512, (w + 1) * 512)
                            den = norm.tile([1, 512], f32, tag="den")
                            nc.scalar.copy(den[:], pvo[HD:HD + 1, wsl])
                            bct = spsum.tile([128, 512], f32, tag="s",
                                             name="bct")
                            bc = bct[0:HD, :]
                            nc.tensor.matmul(bc[:], ones_f32[0:1, :], den[:],
                                             start=True, stop=True)
                            rbc = norm.tile([HD, 512], f32, tag="rbc")
                            nc.vector.reciprocal(rbc[:], bc[:])
                            nc.vector.tensor_tensor(
                                out=houtb[hs, b, wsl], in0=pvo[0:HD, wsl],
                                in1=rbc[:], op=MULT)

                        # global rows (i = 0, 1): dense attention, overwrite
                        NG = NUM_GLOBAL
                        q_rhs = qT[hs, base:base + NG]
                        gsc = spsum.tile([128, 512], f32, tag="s")
                        for jc in range(NT):
                            nc.tensor.matmul(
                                gsc[:, jc * NG:(jc + 1) * NG],
                                kT[hs, base + jc * 128:base + (jc + 1) * 128],
                                q_rhs, start=True, stop=True)
                        pg = norm.tile([128, NT * NG], bf16, tag="pgr")
                        nc.scalar.activation(pg[:], gsc[:, :NT * NG], EXP)
                        pvgt = spsum.tile([128, 512], f32, tag="s",
                                          name="pvgt")
                        pvg = pvgt[0:HD + 1, 0:NG]
                        for jc in range(NT):
                            nc.tensor.matmul(pvg[:],
                                             vaug[b][h][:, jc, 0:HD + 1],
                                             pg[:, jc * NG:(jc + 1) * NG],
                                             start=(jc == 0),
                                             stop=(jc == NT - 1))
                        deng = norm.tile([1, NG], f32, tag="deng")
                        nc.scalar.copy(deng[:], pvg[HD:HD + 1, :])
                        bcgt = spsum.tile([128, 512], f32, tag="s",
                                          name="bcgt")
                        bcg = bcgt[0:HD, 0:NG]
                        nc.tensor.matmul(bcg[:], ones_f32[0:1, :], deng[:],
                                         start=True, stop=True)
                        rbcg = norm.tile([HD, NG], f32, tag="rbcg")
                        nc.vector.reciprocal(rbcg[:], bcg[:])
                        nc.vector.tensor_tensor(
                            out=houtb[hs, b, 0:NG], in0=pvg[0:HD, :],
                            in1=rbcg[:], op=MULT)

        if "D" not in STAGES:
            return
        # ---- stage D: output projection, both heads contracted (K=128)
        with tc.tile_pool(name="osb", bufs=3) as opool, \
                tc.tile_pool(name="opsum", bufs=4, space="PSUM") as opsum:
            for b in range(B):
                for k in range(8):
                    ob = opool.tile([128, S], bf16, tag="ob")
                    for w in range(4):
                        po = opsum.tile([128, 512], f32, tag="po")
                        nc.tensor.matmul(
                            po[:], wo_sb[:, k * 128:(k + 1) * 128],
                            houtb[:, b, w * 512:(w + 1) * 512],
                            start=True, stop=True)
                        if w % 2 == 0:
                            nc.vector.tensor_copy(ob[:, w * 512:(w + 1) * 512],
                                                  po[:])
                        else:
                            nc.scalar.copy(ob[:, w * 512:(w + 1) * 512], po[:])
                    nc.sync.dma_start(
                        t_["outT"][k * 128:(k + 1) * 128,
                                   b * S:(b + 1) * S], ob[:])


# ---------------------------------------------------------------- execution
_NC_CACHE = None


def _get_nc():
    global _NC_CACHE
    if _NC_CACHE is None:
        _NC_CACHE = build_kernel()
    return _NC_CACHE


def _install_axon_trace_shim():
    import sys
    import types

    if "antenv.axon_hooks" in sys.modules:
        return
    mod = types.ModuleType("antenv.axon_hooks")
    mod._hook = None
    mod.set_axon_ntff_profile_hook = lambda h: setattr(mod, "_hook", h)
    mod.get_axon_ntff_profile_hook = lambda: mod._hook
    sys.modules["antenv.axon_hooks"] = mod
    try:
        import antenv
        antenv.axon_hooks = mod
        from trn_agent_boot.trn_boot import _ntff_profile_via_ctypes
        mod._hook = _ntff_profile_via_ctypes("/opt/axon/libaxon_pjrt.so")
    except Exception:
        pass


def run_on_hw(in_maps, trace=False, trace_kwargs=None):
    import os
    _install_axon_trace_shim()
    from concourse import bass_utils
    bass_utils.upload_artifacts = lambda tmpdir: f"local:{tmpdir}"
    if os.environ.get("K_LDW_OPT") and not getattr(bass_utils, "_ldw_patched", 0):
        _orig_rc = bass_utils.run_command
        def _rc(cmd, **kw):
            cmd = ["--enable-ldw-opt=true" if c == "--enable-ldw-opt=false"
                   else c for c in cmd]
            return _orig_rc(cmd, **kw)
        bass_utils.run_command = _rc
        bass_utils._ldw_patched = 1

    nc = _get_nc()
    res = bass_utils.run_bass_kernel_spmd(
        nc, in_maps, core_ids=list(range(N_CORES)), trace=trace,
        trace_kwargs=trace_kwargs or {})
    return res.results, res


def kernel(**inputs):
    in_maps = make_in_maps(inputs)
    results, _ = run_on_hw(in_maps, trace=False)
    out = np.zeros((D, R), dtype=np.float32)
    for c in range(N_CORES):
        out += results[c]["outT"]
    out = out.T + np.asarray(inputs["o_b"], dtype=np.float32)[None, :]
    return np.ascontiguousarray(out.reshape(B, S, D))


# revision 3
# speedup vs baseline: 1.0685x; 1.0685x over previous
"""BigBird sparse attention on 8 Trainium2 NeuronCores — v2.

Sharding: 16 heads across 8 cores (2 heads/core, both batches per core).
Per core: q/k/v projections for its 2 heads, block-sparse BigBird attention,
partial output projection (contracting both heads at once, K=128), output
written as outT [D, R] bf16; host sums the 8 partials (transposed) + o_b.

v2 changes vs baseline:
- Random-key K/V fetched with ROW-mode dma_gather only (no transpose-mode
  SWDGE gathers); K chunks transposed to column layout on the PE.
- V gathered with a denominator-rider column (1.0) per (b, h) so every PV
  matmul is a single M=65 accumulation (no separate denominator matmuls).
- QK/PV restructured j-major: global key block j=0 processed with N=512
  matmuls over all queries; band key blocks j>=1 with N<=384 matmuls over
  the 3 adjacent query blocks; PV accumulates into one [65, S] PSUM region
  per (b, h) using partial-region start/stop groups.
- Normalization via tensor_tensor divide against a PE-broadcast denominator.
- Output projection contracts both heads per matmul (K=128) into oT layout.
"""

import math
import numpy as np

# ---------------------------------------------------------------- constants
B = 2
S = 2048
D = 1024
H = 16
HD = 64
NUM_GLOBAL = 2
NUM_RANDOM = 3
WINDOW = 3

N_CORES = 8
HPC = H // N_CORES          # heads per core = 2
HD2 = HPC * HD              # 128 = head-dim slice per core
R = B * S                   # 4096 flattened rows
NT = S // 128               # 16 i-blocks / j-blocks
NRG = NUM_RANDOM * 128      # gathered random keys per i-block = 384
NIDX = NT * NRG             # 6144 gather indices (each row holds both b)
NCH = NIDX // 128           # 48 gathered chunks of 128 keys

INV_SQRT_HD = 1.0 / math.sqrt(float(HD))

# j-major band spans: i-block range [TLO[j], THI[j]] uses key block j as a
# band chunk. j=0 is the global pass (all i-blocks).
TLO = [0] + [max(j - 1, 0) for j in range(1, NT)]
THI = [NT - 1] + [min(j + 1, NT - 1) for j in range(1, NT)]
BSPAN = [128 * (THI[j] - TLO[j] + 1) for j in range(NT)]  # cols; j=0: 2048

# mask layout: [glob 2048][band j=1..15][rand 48*128]
MOFF_BAND = [0] * NT
off = BSPAN[0]
for j in range(1, NT):
    MOFF_BAND[j] = off
    off += BSPAN[j]
MOFF_RAND = off
NMASK = off + NCH * 128


# ---------------------------------------------------------------- host prep
def _build_ref_mask(random_indices):
    i = np.arange(S)[:, None]
    j = np.arange(S)[None, :]
    glob = (i < NUM_GLOBAL) | (j < NUM_GLOBAL)
    win = np.abs(i - j) <= WINDOW
    rand = np.zeros((S, S), dtype=bool)
    rows = np.repeat(np.arange(S), NUM_RANDOM)
    rand[rows, random_indices.reshape(-1)] = True
    return glob | win | rand


def _host_masks_and_idx(random_indices):
    """j-major band masks + rand chunk masks [128, NMASK] bf16, gather idx."""
    import ml_dtypes

    ri = np.asarray(random_indices).astype(np.int64)
    M = _build_ref_mask(ri)

    masks = np.zeros((128, NMASK), dtype=np.float32)
    # global + band: exact reference mask restricted to (j-block, i-span),
    # transposed to [j-key, i].
    for j in range(NT):
        moff = MOFF_BAND[j]
        ilo, ihi = TLO[j] * 128, (THI[j] + 1) * 128
        sub = M[ilo:ihi, j * 128:(j + 1) * 128]  # [i, j]
        masks[:, moff:moff + (ihi - ilo)] = sub.T.astype(np.float32)
    # rand chunks: c = 3t + g covers key n = 128g + p of i-block t
    for t in range(NT):
        covered = {jb for jb in (t - 1, t, t + 1) if 0 <= jb < NT} | {0}
        for g in range(NUM_RANDOM):
            c = NUM_RANDOM * t + g
            blk = np.zeros((128, 128), dtype=np.float32)
            for p in range(128):
                n = 128 * g + p
                il = n // NUM_RANDOM
                m = n % NUM_RANDOM
                i_glob = t * 128 + il
                r_ = ri[i_glob, m]
                if r_ // 128 in covered:
                    continue
                if any(ri[i_glob, mm] == r_ for mm in range(m)):
                    continue
                blk[p, il] = 1.0
            masks[:, MOFF_RAND + c * 128:MOFF_RAND + (c + 1) * 128] = blk
    masks_bf16 = masks.astype(ml_dtypes.bfloat16)

    # gather indices: flat order n = t*384 + 128*g + p
    n = np.arange(NIDX)
    t_of = n // NRG
    g_of = (n % NRG) // 128
    p_of = n % 128
    nn = 128 * g_of + p_of
    il = nn // NUM_RANDOM
    m = nn % NUM_RANDOM
    j_of = ri[t_of * 128 + il, m]
    vals = j_of.astype(np.int16)
    a16 = np.zeros((16, NIDX // 16), dtype=np.int16)
    a16[n % 16, n // 16] = vals
    return masks_bf16, np.tile(a16, (8, 1))


def make_in_maps(inputs):
    import ml_dtypes

    x = np.asarray(inputs["x"], dtype=np.float32)
    ri = np.asarray(inputs["random_indices"])
    q_w = np.asarray(inputs["q_w"], dtype=np.float32)
    k_w = np.asarray(inputs["k_w"], dtype=np.float32)
    v_w = np.asarray(inputs["v_w"], dtype=np.float32)
    o_w = np.asarray(inputs["o_w"], dtype=np.float32)
    q_b = np.asarray(inputs["q_b"], dtype=np.float32)
    k_b = np.asarray(inputs["k_b"], dtype=np.float32)
    v_b = np.asarray(inputs["v_b"], dtype=np.float32)

    xT = np.ascontiguousarray(x.reshape(R, D).T).astype(ml_dtypes.bfloat16)
    masks, gidx = _host_masks_and_idx(ri)

    q_w = q_w * INV_SQRT_HD
    q_b = q_b * INV_SQRT_HD

    in_maps = []
    for c in range(N_CORES):
        sl = slice(HD2 * c, HD2 * (c + 1))
        in_maps.append({
            "xT": xT,
            "wq": np.ascontiguousarray(q_w[sl, :].T).astype(ml_dtypes.bfloat16),
            "wk": np.ascontiguousarray(k_w[sl, :].T).astype(ml_dtypes.bfloat16),
            "wv": np.ascontiguousarray(v_w[sl, :].T).astype(ml_dtypes.bfloat16),
            "bq": np.ascontiguousarray(q_b[sl, None]),
            "bk": np.ascontiguousarray(k_b[sl, None]),
            "bv": np.ascontiguousarray(v_b[sl, None]),
            "wo": np.ascontiguousarray(o_w[:, sl].T).astype(ml_dtypes.bfloat16),
            "masks": masks,
            "gidx": gidx,
        })
    return in_maps


# ---------------------------------------------------------------- device IR
def build_kernel():
    import concourse.tile as tile
    from concourse import bacc, mybir

    nc = bacc.Bacc("TRN2", target_bir_lowering=False, debug=False,
                   num_swdge_queues=4)
    f32 = mybir.dt.float32
    bf16 = mybir.dt.bfloat16
    i16 = mybir.dt.int16

    t_ = dict(
        xT=nc.dram_tensor("xT", [D, R], bf16, kind="ExternalInput").ap(),
        wq=nc.dram_tensor("wq", [D, HD2], bf16, kind="ExternalInput").ap(),
        wk=nc.dram_tensor("wk", [D, HD2], bf16, kind="ExternalInput").ap(),
        wv=nc.dram_tensor("wv", [D, HD2], bf16, kind="ExternalInput").ap(),
        bq=nc.dram_tensor("bq", [HD2, 1], f32, kind="ExternalInput").ap(),
        bk=nc.dram_tensor("bk", [HD2, 1], f32, kind="ExternalInput").ap(),
        bv=nc.dram_tensor("bv", [HD2, 1], f32, kind="ExternalInput").ap(),
        wo=nc.dram_tensor("wo", [HD2, D], bf16, kind="ExternalInput").ap(),
        masks=nc.dram_tensor("masks", [128, NMASK], bf16,
                             kind="ExternalInput").ap(),
        gidx=nc.dram_tensor("gidx", [128, NIDX // 16], i16,
                            kind="ExternalInput").ap(),
        outT=nc.dram_tensor("outT", [D, R], bf16, kind="ExternalOutput").ap(),
        v_stage=nc.dram_tensor("v_stage", [S, B, HPC, 128], bf16).ap(),
    )

    with tile.TileContext(nc) as tc:
        _build_tc(nc, tc, t_)
    nc.compile()
    return nc


def _build_tc(nc, tc, t_):
    import os
    from contextlib import ExitStack
    STAGES = os.environ.get("K_STAGES", "ABGCD")

    import concourse.bass as bass
    from concourse import masks as cmasks, mybir

    f32 = mybir.dt.float32
    bf16 = mybir.dt.bfloat16
    EXP = mybir.ActivationFunctionType.Exp
    MULT = mybir.AluOpType.mult
    DIV = mybir.AluOpType.divide

    with ExitStack() as ctx:
        const = ctx.enter_context(tc.tile_pool(name="const", bufs=1))
        proj = ctx.enter_context(tc.tile_pool(name="proj", bufs=1))
        persist = ctx.enter_context(tc.tile_pool(name="persist", bufs=1))

        # ---- constants
        ident = const.tile([128, 128], bf16)
        cmasks.make_identity(nc, ident[:])
        ones_f32 = const.tile([128, HD], f32)
        nc.vector.memset(ones_f32[:], 1.0)

        wq_sb = const.tile([128, 8, HD2], bf16)
        wk_sb = const.tile([128, 8, HD2], bf16)
        wv_sb = const.tile([128, 8, HD2], bf16)
        for w_sb, w_d in ((wq_sb, t_["wq"]), (wk_sb, t_["wk"]),
                          (wv_sb, t_["wv"])):
            nc.sync.dma_start(w_sb[:], w_d.rearrange("(c p) m -> p c m", p=128))
        bq_sb = const.tile([HD2, 1], f32)
        bk_sb = const.tile([HD2, 1], f32)
        bv_sb = const.tile([HD2, 1], f32)
        nc.sync.dma_start(bq_sb[:], t_["bq"])
        nc.sync.dma_start(bk_sb[:], t_["bk"])
        nc.sync.dma_start(bv_sb[:], t_["bv"])
        wo_sb = const.tile([HD2, D], bf16)
        nc.sync.dma_start(wo_sb[:], t_["wo"])
        mask_sb = const.tile([128, NMASK], bf16)
        nc.sync.dma_start(mask_sb[:], t_["masks"])
        gidx_sb = const.tile([128, NIDX // 16], mybir.dt.int16)
        nc.sync.dma_start(gidx_sb[:], t_["gidx"])

        # persistent activations
        qT = proj.tile([128, R], bf16)
        kT = proj.tile([128, R], bf16)
        # both heads' attention outputs stacked: h0 -> partitions 0..63
        houtb = proj.tile([128, B, S], bf16)
        # v rows + ones rider at col HD, per (b, h)
        vaug = [[persist.tile([128, NT, 128], bf16, name=f"vaug{b}{h}")
                 for h in range(HPC)] for b in range(B)]
        for b in range(B):
            for h in range(HPC):
                nc.vector.memset(vaug[b][h][:, :, HD + 1:], 0.0)
                nc.vector.memset(vaug[b][h][:, :, HD:HD + 1], 1.0)
        # gathered tensors; vselaug free layout per chunk:
        # [b0h0 d0..63, one, pad*63, b0h1 ..., b1h0 ..., b1h1 ...]
        # kselT/vselaug are whole-tile copies made after ALL gathers land —
        # the copy's whole-tile read waits on final (order-independent) DMASW
        # sem counts, making the 4-queue gather spread race-free. vselaug is
        # compacted to the 65 used cols per (b, h).
        vselaug = persist.tile([128, NCH, B * HPC, HD + 1], bf16)
        kselT = persist.tile([128, B, NIDX // 768, 768], bf16)
        krows_p = [persist.tile([128, NT, HD2], bf16, name=f"krowsp{b}")
                   for b in range(B)]

        # ---- stage A: projections -> qT/kT/vT [128(hd2), R] bf16
        with tc.tile_pool(name="stgA", bufs=1) as stga, \
                tc.tile_pool(name="xstream", bufs=2) as xpool, \
                tc.tile_pool(name="ppsum", bufs=3, space="PSUM") as ppsum:
            vT = stga.tile([128, R], bf16)
            xT_r = t_["xT"].rearrange("(c p) r -> p c r", p=128)
            for rt in range(R // 512):
                xt = xpool.tile([128, 8, 512], bf16)
                nc.sync.dma_start(xt[:], xT_r[:, :, bass.ts(rt, 512)])
                for dst, w_sb, b_sb in ((qT, wq_sb, bq_sb), (kT, wk_sb, bk_sb),
                                        (vT, wv_sb, bv_sb)):
                    ps = ppsum.tile([128, 512], f32, tag="ps")
                    for dc in range(8):
                        nc.tensor.matmul(ps[:], w_sb[:, dc, :], xt[:, dc, :],
                                         start=(dc == 0), stop=(dc == 7))
                    nc.vector.tensor_scalar_add(dst[:, bass.ts(rt, 512)],
                                                ps[:], b_sb[:])

            # ---- stage B: k/v row-major staging (PE transposes)
            with tc.tile_pool(name="stgB", bufs=1) as stg, \
                    tc.tile_pool(name="tpsum", bufs=4, space="PSUM") as tpsum:
                for b in range(B):
                    krows = krows_p[b]
                    for jc in range(NT):
                        csl = slice(b * S + jc * 128, b * S + (jc + 1) * 128)
                        psk = tpsum.tile([128, 128], bf16, tag="tp")
                        nc.tensor.transpose(psk[:], kT[:, csl], ident[:])
                        nc.vector.tensor_copy(krows[:, jc, :], psk[:])
                        psv = tpsum.tile([128, 128], bf16, tag="tp")
                        nc.tensor.transpose(psv[:], vT[:, csl], ident[:])
                        nc.vector.tensor_copy(vaug[b][0][:, jc, 0:HD],
                                              psv[:, 0:HD])
                        nc.vector.tensor_copy(vaug[b][1][:, jc, 0:HD],
                                              psv[:, HD:HD2])
                    for h in range(HPC):
                        nc.sync.dma_start(
                            t_["v_stage"][:, b, h, :].rearrange(
                                "(c p) e -> p c e", p=128),
                            vaug[b][h][:, :, :])

        # ---- gathers: V rows from DRAM; K via SBUF-source transpose
        # gathers from krows (column-layout output, no PE transposes).
        # 768 idx per call (49 SWDGE FIFO entries), spread over 4 queues.
        CR = 768
        NCALL = NIDX // CR  # 8
        v_src = t_["v_stage"].rearrange("j b h e -> j (b h e)")
        with tc.tile_pool(name="graw", bufs=1) as graw:
            kselT0 = graw.tile([128, B, NIDX // 768, 768], bf16)
            vselaug0 = graw.tile([128, NCH, B * HPC * 128], bf16)
            # queue = emission_index % 4 keeps each DMASW sem lane (index % 8)
            # on a single queue, preserving per-lane completion order.
            gq = [0]

            def nextq():
                q = gq[0] % 4
                gq[0] += 1
                return q

            for u in range(NCALL):
                isl = slice(u * (CR // 16), (u + 1) * (CR // 16))
                for b in range(B):
                    nc.gpsimd.dma_gather(
                        kselT0[:, b, u:u + 1, :], krows_p[b][:],
                        gidx_sb[:, isl], CR, CR, HD2, transpose=True,
                        queue_num=nextq(),
                        sbuf_tokens_per_rank=128,
                        sbuf_free_dim_per_rank=HD2 * 2)
            nc.vector.tensor_copy(kselT[:], kselT0[:])
            for u in range(NCALL):
                isl = slice(u * (CR // 16), (u + 1) * (CR // 16))
                nc.gpsimd.dma_gather(
                    vselaug0[:, u * (CR // 128):(u + 1) * (CR // 128), :],
                    v_src, gidx_sb[:, isl], CR, CR, B * HPC * 128,
                    transpose=False, queue_num=nextq())
            v0v = vselaug0[:].rearrange("p c (x e) -> p c x e", e=128)
            nc.scalar.copy(vselaug[:], v0v[:, :, :, 0:HD + 1])

            if "C" not in STAGES:
                return
            # ---- stage C: attention per (b, h)
            with tc.tile_pool(name="pglob", bufs=2) as pgpool, \
                    tc.tile_pool(name="pband", bufs=6) as pbpool, \
                    tc.tile_pool(name="norm", bufs=4) as norm, \
                    tc.tile_pool(name="spsum", bufs=3, space="PSUM") as spsum, \
                    tc.tile_pool(name="vpsum", bufs=1, space="PSUM") as vpsum:
                for b in range(B):
                    for h in range(HPC):
                        hs = slice(HD * h, HD * (h + 1))
                        base = b * S
                        pvo = vpsum.tile([HD + 1, S], f32, tag="pv")

                        # work items: QK(j) producing p, then PV(j) consuming
                        # it one step behind, to keep the PE stream dense.
                        p_glob = pgpool.tile([128, S], bf16, tag="pg")
                        p_band = {}
                        p_rand = {}

                        def qk_glob():
                            for w in range(4):
                                ssc = spsum.tile([128, 512], f32, tag="s")
                                nc.tensor.matmul(
                                    ssc[:], kT[hs, base:base + 128],
                                    qT[hs, base + w * 512:base + (w + 1) * 512],
                                    start=True, stop=True)
                                nc.scalar.activation(
                                    p_glob[:, w * 512:(w + 1) * 512], ssc[:],
                                    EXP)
                            nc.vector.tensor_tensor(
                                out=p_glob[:], in0=p_glob[:],
                                in1=mask_sb[:, 0:S], op=MULT)

                        def qk_band(j):
                            span = BSPAN[j]
                            ilo = TLO[j] * 128
                            ssc = spsum.tile([128, 512], f32, tag="s")
                            nc.tensor.matmul(
                                ssc[:, 0:span],
                                kT[hs, base + j * 128:base + (j + 1) * 128],
                                qT[hs, base + ilo:base + ilo + span],
                                start=True, stop=True)
                            pb = pbpool.tile([128, 384], bf16, tag="pb",
                                             name=f"pb{b}{h}_{j}")
                            p_band[j] = pb
                            nc.scalar.activation(pb[:, 0:span], ssc[:, 0:span],
                                                 EXP)
                            moff = MOFF_BAND[j]
                            nc.vector.tensor_tensor(
                                out=pb[:, 0:span], in0=pb[:, 0:span],
                                in1=mask_sb[:, moff:moff + span], op=MULT)

                        def qk_rand(t):
                            ssc = spsum.tile([128, 512], f32, tag="s")
                            for g in range(NUM_RANDOM):
                                n0 = t * NRG + g * 128
                                nc.tensor.matmul(
                                    ssc[:, g * 128:(g + 1) * 128],
                                    kselT[hs, b, n0 // 768,
                                          n0 % 768:n0 % 768 + 128],
                                    qT[hs, base + t * 128:base + (t + 1) * 128],
                                    start=True, stop=True)
                            pr = pbpool.tile([128, 384], bf16, tag="pr",
                                             name=f"pr{b}{h}_{t}")
                            p_rand[t] = pr
                            nc.scalar.activation(pr[:], ssc[:, 0:NRG], EXP)
                            m0 = MOFF_RAND + t * NRG
                            nc.vector.tensor_tensor(
                                out=pr[:], in0=pr[:],
                                in1=mask_sb[:, m0:m0 + NRG], op=MULT)

                        def pv_glob():
                            # init whole [65, S] region (global keys attend
                            # to every query)
                            for w in range(4):
                                nc.tensor.matmul(
                                    pvo[:, w * 512:(w + 1) * 512],
                                    vaug[b][h][:, 0, 0:HD + 1],
                                    p_glob[:, w * 512:(w + 1) * 512],
                                    start=True, stop=False,
                                    skip_group_check=True)

                        def pv_band(j):
                            span = BSPAN[j]
                            ilo = TLO[j] * 128
                            # split at PSUM bank (512-col) boundaries
                            lo = ilo
                            while lo < ilo + span:
                                hi = min(ilo + span, (lo // 512 + 1) * 512)
                                nc.tensor.matmul(
                                    pvo[:, lo:hi],
                                    vaug[b][h][:, j, 0:HD + 1],
                                    p_band[j][:, lo - ilo:hi - ilo],
                                    start=False, stop=False,
                                    skip_group_check=True)
                                lo = hi
                            del p_band[j]

                        def pv_rand(t):
                            for g in range(NUM_RANDOM):
                                c = NUM_RANDOM * t + g
                                nc.tensor.matmul(
                                    pvo[:, t * 128:(t + 1) * 128],
                                    vselaug[:, c, b * HPC + h, :],
                                    p_rand[t][:, g * 128:(g + 1) * 128],
                                    start=False, stop=(g == NUM_RANDOM - 1),
                                    skip_group_check=True)
                            del p_rand[t]

                        # software-pipelined emission: QK one step ahead of PV
                        work_qk = ([qk_glob]
                                   + [lambda j=j: qk_band(j)
                                      for j in range(1, NT)]
                                   + [lambda t=t: qk_rand(t)
                                      for t in range(NT)])
                        work_pv = ([pv_glob]
                                   + [lambda j=j: pv_band(j)
                                      for j in range(1, NT)]
                                   + [lambda t=t: pv_rand(t)
                                      for t in range(NT)])
                        work_qk[0]()
                        if len(work_qk) > 1:
                            work_qk[1]()
                        for i in range(len(work_pv)):
                            if i + 2 < len(work_qk):
                                work_qk[i + 2]()
                            work_pv[i]()

                        # normalize: hout = pv / den per 512-col group
                        for w in range(4):
                            wsl = slice(w * 512, (w + 1) * 512)
                            den = norm.tile([1, 512], f32, tag="den")
                            nc.scalar.copy(den[:], pvo[HD:HD + 1, wsl])
                            bct = spsum.tile([128, 512], f32, tag="s",
                                             name="bct")
                            bc = bct[0:HD, :]
                            nc.tensor.matmul(bc[:], ones_f32[0:1, :], den[:],
                                             start=True, stop=True)
                            rbc = norm.tile([HD, 512], f32, tag="rbc")
                            nc.vector.reciprocal(rbc[:], bc[:])
                            nc.vector.tensor_tensor(
                                out=houtb[hs, b, wsl], in0=pvo[0:HD, wsl],
                                in1=rbc[:], op=MULT)

                        # global rows (i = 0, 1): dense attention, overwrite
                        NG = NUM_GLOBAL
                        q_rhs = qT[hs, base:base + NG]
                        gsc = spsum.tile([128, 512], f32, tag="s")
                        for jc in range(NT):
                            nc.tensor.matmul(
                                gsc[:, jc * NG:(jc + 1) * NG],
                                kT[hs, base + jc * 128:base + (jc + 1) * 128],
                                q_rhs, start=True, stop=True)
                        pg = norm.tile([128, NT * NG], bf16, tag="pgr")
                        nc.scalar.activation(pg[:], gsc[:, :NT * NG], EXP)
                        pvgt = spsum.tile([128, 512], f32, tag="s",
                                          name="pvgt")
                        pvg = pvgt[0:HD + 1, 0:NG]
                        for jc in range(NT):
                            nc.tensor.matmul(pvg[:],
                                             vaug[b][h][:, jc, 0:HD + 1],
                                             pg[:, jc * NG:(jc + 1) * NG],
                                             start=(jc == 0),
                                             stop=(jc == NT - 1))
                        deng = norm.tile([1, NG], f32, tag="deng")
                        nc.scalar.copy(deng[:], pvg[HD:HD + 1, :])
                        bcgt = spsum.tile([128, 512], f32, tag="s",
                                          name="bcgt")
                        bcg = bcgt[0:HD, 0:NG]
                        nc.tensor.matmul(bcg[:], ones_f32[0:1, :], deng[:],
                                         start=True, stop=True)
                        rbcg = norm.tile([HD, NG], f32, tag="rbcg")
                        nc.vector.reciprocal(rbcg[:], bcg[:])
                        nc.vector.tensor_tensor(
                            out=houtb[hs, b, 0:NG], in0=pvg[0:HD, :],
                            in1=rbcg[:], op=MULT)

        if "D" not in STAGES:
            return
        # ---- stage D: output projection, both heads contracted (K=128)
        with tc.tile_pool(name="osb", bufs=3) as opool, \
                tc.tile_pool(name="opsum", bufs=4, space="PSUM") as opsum:
            for b in range(B):
                for k in range(8):
                    ob = opool.tile([128, S], bf16, tag="ob")
                    for w in range(4):
                        po = opsum.tile([128, 512], f32, tag="po")
                        nc.tensor.matmul(
                            po[:], wo_sb[:, k * 128:(k + 1) * 128],
                            houtb[:, b, w * 512:(w + 1) * 512],
                            start=True, stop=True)
                        if w % 2 == 0:
                            nc.vector.tensor_copy(ob[:, w * 512:(w + 1) * 512],
                                                  po[:])
                        else:
                            nc.scalar.copy(ob[:, w * 512:(w + 1) * 512], po[:])
                    nc.sync.dma_start(
                        t_["outT"][k * 128:(k + 1) * 128,
                                   b * S:(b + 1) * S], ob[:])


# ---------------------------------------------------------------- execution
_NC_CACHE = None


def _get_nc():
    global _NC_CACHE
    if _NC_CACHE is None:
        _NC_CACHE = build_kernel()
    return _NC_CACHE


def _install_axon_trace_shim():
    import sys
    import types

    if "antenv.axon_hooks" in sys.modules:
        return
    mod = types.ModuleType("antenv.axon_hooks")
    mod._hook = None
    mod.set_axon_ntff_profile_hook = lambda h: setattr(mod, "_hook", h)
    mod.get_axon_ntff_profile_hook = lambda: mod._hook
    sys.modules["antenv.axon_hooks"] = mod
    try:
        import antenv
        antenv.axon_hooks = mod
        from trn_agent_boot.trn_boot import _ntff_profile_via_ctypes
        mod._hook = _ntff_profile_via_ctypes("/opt/axon/libaxon_pjrt.so")
    except Exception:
        pass


def run_on_hw(in_maps, trace=False, trace_kwargs=None):
    import os
    _install_axon_trace_shim()
    from concourse import bass_utils
    bass_utils.upload_artifacts = lambda tmpdir: f"local:{tmpdir}"
    if os.environ.get("K_LDW_OPT") and not getattr(bass_utils, "_ldw_patched", 0):
        _orig_rc = bass_utils.run_command
        def _rc(cmd, **kw):
            cmd = ["--enable-ldw-opt=true" if c == "--enable-ldw-opt=false"
                   else c for c in cmd]
            return _orig_rc(cmd, **kw)
        bass_utils.run_command = _rc
        bass_utils._ldw_patched = 1

    nc = _get_nc()
    res = bass_utils.run_bass_kernel_spmd(
        nc, in_maps, core_ids=list(range(N_CORES)), trace=trace,
        trace_kwargs=trace_kwargs or {})
    return res.results, res


def kernel(**inputs):
    in_maps = make_in_maps(inputs)
    results, _ = run_on_hw(in_maps, trace=False)
    out = np.zeros((D, R), dtype=np.float32)
    for c in range(N_CORES):
        out += results[c]["outT"]
    out = out.T + np.asarray(inputs["o_b"], dtype=np.float32)[None, :]
    return np.ascontiguousarray(out.reshape(B, S, D))


# revision 4
# speedup vs baseline: 1.0961x; 1.0259x over previous
"""BigBird sparse attention on 8 Trainium2 NeuronCores — v2.

Sharding: 16 heads across 8 cores (2 heads/core, both batches per core).
Per core: q/k/v projections for its 2 heads, block-sparse BigBird attention,
partial output projection (contracting both heads at once, K=128), output
written as outT [D, R] bf16; host sums the 8 partials (transposed) + o_b.

v2 changes vs baseline:
- Random-key K/V fetched with ROW-mode dma_gather only (no transpose-mode
  SWDGE gathers); K chunks transposed to column layout on the PE.
- V gathered with a denominator-rider column (1.0) per (b, h) so every PV
  matmul is a single M=65 accumulation (no separate denominator matmuls).
- QK/PV restructured j-major: global key block j=0 processed with N=512
  matmuls over all queries; band key blocks j>=1 with N<=384 matmuls over
  the 3 adjacent query blocks; PV accumulates into one [65, S] PSUM region
  per (b, h) using partial-region start/stop groups.
- Normalization via tensor_tensor divide against a PE-broadcast denominator.
- Output projection contracts both heads per matmul (K=128) into oT layout.
"""

import math
import numpy as np

# ---------------------------------------------------------------- constants
B = 2
S = 2048
D = 1024
H = 16
HD = 64
NUM_GLOBAL = 2
NUM_RANDOM = 3
WINDOW = 3

N_CORES = 8
HPC = H // N_CORES          # heads per core = 2
HD2 = HPC * HD              # 128 = head-dim slice per core
R = B * S                   # 4096 flattened rows
NT = S // 128               # 16 i-blocks / j-blocks
NRG = NUM_RANDOM * 128      # gathered random keys per i-block = 384
NIDX = NT * NRG             # 6144 gather indices (each row holds both b)
NCH = NIDX // 128           # 48 gathered chunks of 128 keys

INV_SQRT_HD = 1.0 / math.sqrt(float(HD))

# j-major band spans: i-block range [TLO[j], THI[j]] uses key block j as a
# band chunk. j=0 is the global pass (all i-blocks).
TLO = [0] + [max(j - 1, 0) for j in range(1, NT)]
THI = [NT - 1] + [min(j + 1, NT - 1) for j in range(1, NT)]
BSPAN = [128 * (THI[j] - TLO[j] + 1) for j in range(NT)]  # cols; j=0: 2048

# mask layout: [glob 2048][band j=1..15][rand 48*128]
MOFF_BAND = [0] * NT
off = BSPAN[0]
for j in range(1, NT):
    MOFF_BAND[j] = off
    off += BSPAN[j]
MOFF_RAND = off
NMASK = off + NCH * 128


# ---------------------------------------------------------------- host prep
def _build_ref_mask(random_indices):
    i = np.arange(S)[:, None]
    j = np.arange(S)[None, :]
    glob = (i < NUM_GLOBAL) | (j < NUM_GLOBAL)
    win = np.abs(i - j) <= WINDOW
    rand = np.zeros((S, S), dtype=bool)
    rows = np.repeat(np.arange(S), NUM_RANDOM)
    rand[rows, random_indices.reshape(-1)] = True
    return glob | win | rand


def _host_masks_and_idx(random_indices):
    """j-major band masks + rand chunk masks [128, NMASK] bf16, gather idx."""
    import ml_dtypes

    ri = np.asarray(random_indices).astype(np.int64)
    M = _build_ref_mask(ri)

    masks = np.zeros((128, NMASK), dtype=np.float32)
    # global + band: exact reference mask restricted to (j-block, i-span),
    # transposed to [j-key, i].
    for j in range(NT):
        moff = MOFF_BAND[j]
        ilo, ihi = TLO[j] * 128, (THI[j] + 1) * 128
        sub = M[ilo:ihi, j * 128:(j + 1) * 128]  # [i, j]
        masks[:, moff:moff + (ihi - ilo)] = sub.T.astype(np.float32)
    # rand chunks: c = 3t + g covers key n = 128g + p of i-block t
    for t in range(NT):
        covered = {jb for jb in (t - 1, t, t + 1) if 0 <= jb < NT} | {0}
        for g in range(NUM_RANDOM):
            c = NUM_RANDOM * t + g
            blk = np.zeros((128, 128), dtype=np.float32)
            for p in range(128):
                n = 128 * g + p
                il = n // NUM_RANDOM
                m = n % NUM_RANDOM
                i_glob = t * 128 + il
                r_ = ri[i_glob, m]
                if r_ // 128 in covered:
                    continue
                if any(ri[i_glob, mm] == r_ for mm in range(m)):
                    continue
                blk[p, il] = 1.0
            masks[:, MOFF_RAND + c * 128:MOFF_RAND + (c + 1) * 128] = blk
    masks_bf16 = masks.astype(ml_dtypes.bfloat16)

    # gather indices: flat order n = t*384 + 128*g + p
    n = np.arange(NIDX)
    t_of = n // NRG
    g_of = (n % NRG) // 128
    p_of = n % 128
    nn = 128 * g_of + p_of
    il = nn // NUM_RANDOM
    m = nn % NUM_RANDOM
    j_of = ri[t_of * 128 + il, m]
    vals = j_of.astype(np.int16)
    a16 = np.zeros((16, NIDX // 16), dtype=np.int16)
    a16[n % 16, n // 16] = vals
    return masks_bf16, np.tile(a16, (8, 1))


def make_in_maps(inputs):
    import ml_dtypes

    x = np.asarray(inputs["x"], dtype=np.float32)
    ri = np.asarray(inputs["random_indices"])
    q_w = np.asarray(inputs["q_w"], dtype=np.float32)
    k_w = np.asarray(inputs["k_w"], dtype=np.float32)
    v_w = np.asarray(inputs["v_w"], dtype=np.float32)
    o_w = np.asarray(inputs["o_w"], dtype=np.float32)
    q_b = np.asarray(inputs["q_b"], dtype=np.float32)
    k_b = np.asarray(inputs["k_b"], dtype=np.float32)
    v_b = np.asarray(inputs["v_b"], dtype=np.float32)

    xT = np.ascontiguousarray(x.reshape(R, D).T).astype(ml_dtypes.bfloat16)
    masks, gidx = _host_masks_and_idx(ri)

    q_w = q_w * INV_SQRT_HD
    q_b = q_b * INV_SQRT_HD

    in_maps = []
    for c in range(N_CORES):
        sl = slice(HD2 * c, HD2 * (c + 1))
        in_maps.append({
            "xT": xT,
            "wq": np.ascontiguousarray(q_w[sl, :].T).astype(ml_dtypes.bfloat16),
            "wk": np.ascontiguousarray(k_w[sl, :].T).astype(ml_dtypes.bfloat16),
            "wv": np.ascontiguousarray(v_w[sl, :].T).astype(ml_dtypes.bfloat16),
            "bq": np.ascontiguousarray(q_b[sl, None]),
            "bk": np.ascontiguousarray(k_b[sl, None]),
            "bv": np.ascontiguousarray(v_b[sl, None]),
            "wo": np.ascontiguousarray(o_w[:, sl].T).astype(ml_dtypes.bfloat16),
            "masks": masks,
            "gidx": gidx,
        })
    return in_maps


# ---------------------------------------------------------------- device IR
def build_kernel():
    import concourse.tile as tile
    from concourse import bacc, mybir

    nc = bacc.Bacc("TRN2", target_bir_lowering=False, debug=False,
                   num_swdge_queues=4)
    f32 = mybir.dt.float32
    bf16 = mybir.dt.bfloat16
    i16 = mybir.dt.int16

    t_ = dict(
        xT=nc.dram_tensor("xT", [D, R], bf16, kind="ExternalInput").ap(),
        wq=nc.dram_tensor("wq", [D, HD2], bf16, kind="ExternalInput").ap(),
        wk=nc.dram_tensor("wk", [D, HD2], bf16, kind="ExternalInput").ap(),
        wv=nc.dram_tensor("wv", [D, HD2], bf16, kind="ExternalInput").ap(),
        bq=nc.dram_tensor("bq", [HD2, 1], f32, kind="ExternalInput").ap(),
        bk=nc.dram_tensor("bk", [HD2, 1], f32, kind="ExternalInput").ap(),
        bv=nc.dram_tensor("bv", [HD2, 1], f32, kind="ExternalInput").ap(),
        wo=nc.dram_tensor("wo", [HD2, D], bf16, kind="ExternalInput").ap(),
        masks=nc.dram_tensor("masks", [128, NMASK], bf16,
                             kind="ExternalInput").ap(),
        gidx=nc.dram_tensor("gidx", [128, NIDX // 16], i16,
                            kind="ExternalInput").ap(),
        outT=nc.dram_tensor("outT", [D, R], bf16, kind="ExternalOutput").ap(),
        v_stage=nc.dram_tensor("v_stage", [S, B, HPC, 128], bf16).ap(),
    )

    with tile.TileContext(nc) as tc:
        _build_tc(nc, tc, t_)
    nc.compile()
    return nc


def _build_tc(nc, tc, t_):
    import os
    from contextlib import ExitStack
    STAGES = os.environ.get("K_STAGES", "ABGCD")

    import concourse.bass as bass
    from concourse import masks as cmasks, mybir

    f32 = mybir.dt.float32
    bf16 = mybir.dt.bfloat16
    EXP = mybir.ActivationFunctionType.Exp
    MULT = mybir.AluOpType.mult
    DIV = mybir.AluOpType.divide

    with ExitStack() as ctx:
        const = ctx.enter_context(tc.tile_pool(name="const", bufs=1))
        proj = ctx.enter_context(tc.tile_pool(name="proj", bufs=1))
        persist = ctx.enter_context(tc.tile_pool(name="persist", bufs=1))

        # ---- constants
        ident = const.tile([128, 128], bf16)
        cmasks.make_identity(nc, ident[:])
        ones_f32 = const.tile([128, HD], f32)
        nc.vector.memset(ones_f32[:], 1.0)

        wq_sb = const.tile([128, 8, HD2], bf16)
        wk_sb = const.tile([128, 8, HD2], bf16)
        wv_sb = const.tile([128, 8, HD2], bf16)
        for w_sb, w_d in ((wq_sb, t_["wq"]), (wk_sb, t_["wk"]),
                          (wv_sb, t_["wv"])):
            nc.sync.dma_start(w_sb[:], w_d.rearrange("(c p) m -> p c m", p=128))
        bq_sb = const.tile([HD2, 1], f32)
        bk_sb = const.tile([HD2, 1], f32)
        bv_sb = const.tile([HD2, 1], f32)
        nc.sync.dma_start(bq_sb[:], t_["bq"])
        nc.sync.dma_start(bk_sb[:], t_["bk"])
        nc.sync.dma_start(bv_sb[:], t_["bv"])
        wo_sb = const.tile([HD2, D], bf16)
        nc.sync.dma_start(wo_sb[:], t_["wo"])
        mask_sb = const.tile([128, NMASK], bf16)
        nc.sync.dma_start(mask_sb[:], t_["masks"])
        gidx_sb = const.tile([128, NIDX // 16], mybir.dt.int16)
        nc.sync.dma_start(gidx_sb[:], t_["gidx"])

        # persistent activations
        qT = proj.tile([128, R], bf16)
        kT = proj.tile([128, R], bf16)
        # both heads' attention outputs stacked: h0 -> partitions 0..63
        houtb = proj.tile([128, B, S], bf16)
        # v rows + ones rider at col HD, per (b, h)
        vaug = [[persist.tile([128, NT, 128], bf16, name=f"vaug{b}{h}")
                 for h in range(HPC)] for b in range(B)]
        for b in range(B):
            for h in range(HPC):
                nc.vector.memset(vaug[b][h][:, :, HD + 1:], 0.0)
                nc.vector.memset(vaug[b][h][:, :, HD:HD + 1], 1.0)
        # gathered tensors; vselaug free layout per chunk:
        # [b0h0 d0..63, one, pad*63, b0h1 ..., b1h0 ..., b1h1 ...]
        # kselT/vselaug are whole-tile copies made after ALL gathers land —
        # the copy's whole-tile read waits on final (order-independent) DMASW
        # sem counts, making the 4-queue gather spread race-free. vselaug is
        # compacted to the 65 used cols per (b, h).
        vselaug = persist.tile([128, NCH, B * HPC, HD + 1], bf16)
        kselT = persist.tile([128, B, NIDX // 768, 768], bf16)
        kselT0 = persist.tile([128, B, NIDX // 768, 768], bf16)
        krows_p = [persist.tile([128, NT, HD2], bf16, name=f"krowsp{b}")
                   for b in range(B)]

        # ---- stage A: projections -> qT/kT/vT [128(hd2), R] bf16
        with tc.tile_pool(name="stgA", bufs=1) as stga, \
                tc.tile_pool(name="xstream", bufs=2) as xpool, \
                tc.tile_pool(name="ppsum", bufs=3, space="PSUM") as ppsum:
            vT = stga.tile([128, R], bf16)
            xT_r = t_["xT"].rearrange("(c p) r -> p c r", p=128)

            def stage_b(b, tpsum):
                # k/v row-major staging for batch b (PE transposes), then
                # this batch's K gathers start immediately.
                krows = krows_p[b]
                for jc in range(NT):
                    csl = slice(b * S + jc * 128, b * S + (jc + 1) * 128)
                    psk = tpsum.tile([128, 128], bf16, tag="tp")
                    nc.tensor.transpose(psk[:], kT[:, csl], ident[:])
                    if jc % 2 == 0:
                        nc.vector.tensor_copy(krows[:, jc, :], psk[:])
                    else:
                        nc.scalar.copy(krows[:, jc, :], psk[:])
                    psv = tpsum.tile([128, 128], bf16, tag="tp")
                    nc.tensor.transpose(psv[:], vT[:, csl], ident[:])
                    nc.vector.tensor_copy(vaug[b][0][:, jc, 0:HD],
                                          psv[:, 0:HD])
                    nc.scalar.copy(vaug[b][1][:, jc, 0:HD],
                                   psv[:, HD:HD2])
                for h in range(HPC):
                    nc.sync.dma_start(
                        t_["v_stage"][:, b, h, :].rearrange(
                            "(c p) e -> p c e", p=128),
                        vaug[b][h][:, :, :])
                for u in range(NIDX // 768):
                    isl = slice(u * (768 // 16), (u + 1) * (768 // 16))
                    nc.gpsimd.dma_gather(
                        kselT0[:, b, u:u + 1, :], krows[:],
                        gidx_sb[:, isl], 768, 768, HD2, transpose=True,
                        queue_num=(u + 4 * b) % 4,
                        sbuf_tokens_per_rank=128,
                        sbuf_free_dim_per_rank=HD2 * 2)

            with tc.tile_pool(name="tpsumE", bufs=2, space="PSUM") as tpsumE:
                for rt in range(R // 512):
                    xt = xpool.tile([128, 8, 512], bf16)
                    nc.sync.dma_start(xt[:], xT_r[:, :, bass.ts(rt, 512)])
                    for dst, w_sb, b_sb in ((qT, wq_sb, bq_sb),
                                            (kT, wk_sb, bk_sb),
                                            (vT, wv_sb, bv_sb)):
                        ps = ppsum.tile([128, 512], f32, tag="ps")
                        for dc in range(8):
                            nc.tensor.matmul(ps[:], w_sb[:, dc, :],
                                             xt[:, dc, :],
                                             start=(dc == 0), stop=(dc == 7))
                        nc.vector.tensor_scalar_add(dst[:, bass.ts(rt, 512)],
                                                    ps[:], b_sb[:])
                    if rt == 3:
                        stage_b(0, tpsumE)
                    elif rt == 7:
                        stage_b(1, tpsumE)

        # ---- gathers: V rows from DRAM; K via SBUF-source transpose
        # gathers from krows (column-layout output, no PE transposes).
        # 768 idx per call (49 SWDGE FIFO entries), spread over 4 queues.
        CR = 768
        NCALL = NIDX // CR  # 8
        v_src = t_["v_stage"].rearrange("j b h e -> j (b h e)")
        with tc.tile_pool(name="graw", bufs=1) as graw:
            vselaug0 = graw.tile([128, NCH, B * HPC * 128], bf16)
            # queue = emission_index % 4 keeps each DMASW sem lane (index % 8)
            # on a single queue, preserving per-lane completion order.
            gq = [0]

            def nextq():
                q = gq[0] % 4
                gq[0] += 1
                return q

            nc.vector.tensor_copy(kselT[:], kselT0[:])
            for u in range(NCALL):
                isl = slice(u * (CR // 16), (u + 1) * (CR // 16))
                nc.gpsimd.dma_gather(
                    vselaug0[:, u * (CR // 128):(u + 1) * (CR // 128), :],
                    v_src, gidx_sb[:, isl], CR, CR, B * HPC * 128,
                    transpose=False, queue_num=nextq())
            v0v = vselaug0[:].rearrange("p c (x e) -> p c x e", e=128)
            nc.scalar.copy(vselaug[:], v0v[:, :, :, 0:HD + 1])

            if "C" not in STAGES:
                return
            # ---- stage C: attention per (b, h)
            with tc.tile_pool(name="pglob", bufs=2) as pgpool, \
                    tc.tile_pool(name="pband", bufs=6) as pbpool, \
                    tc.tile_pool(name="norm", bufs=4) as norm, \
                    tc.tile_pool(name="spsum", bufs=3, space="PSUM") as spsum, \
                    tc.tile_pool(name="vpsum", bufs=1, space="PSUM") as vpsum:
                for b in range(B):
                    for h in range(HPC):
                        hs = slice(HD * h, HD * (h + 1))
                        base = b * S
                        pvo = vpsum.tile([HD + 1, S], f32, tag="pv")

                        # work items: QK(j) producing p, then PV(j) consuming
                        # it one step behind, to keep the PE stream dense.
                        p_glob = pgpool.tile([128, S], bf16, tag="pg")
                        p_band = {}
                        p_rand = {}

                        def qk_glob():
                            for w in range(4):
                                ssc = spsum.tile([128, 512], f32, tag="s")
                                nc.tensor.matmul(
                                    ssc[:], kT[hs, base:base + 128],
                                    qT[hs, base + w * 512:base + (w + 1) * 512],
                                    start=True, stop=True)
                                nc.scalar.activation(
                                    p_glob[:, w * 512:(w + 1) * 512], ssc[:],
                                    EXP)
                            nc.vector.tensor_tensor(
                                out=p_glob[:], in0=p_glob[:],
                                in1=mask_sb[:, 0:S], op=MULT)

                        def qk_band(j):
                            span = BSPAN[j]
                            ilo = TLO[j] * 128
                            ssc = spsum.tile([128, 512], f32, tag="s")
                            nc.tensor.matmul(
                                ssc[:, 0:span],
                                kT[hs, base + j * 128:base + (j + 1) * 128],
                                qT[hs, base + ilo:base + ilo + span],
                                start=True, stop=True)
                            pb = pbpool.tile([128, 384], bf16, tag="pb",
                                             name=f"pb{b}{h}_{j}")
                            p_band[j] = pb
                            nc.scalar.activation(pb[:, 0:span], ssc[:, 0:span],
                                                 EXP)
                            moff = MOFF_BAND[j]
                            nc.vector.tensor_tensor(
                                out=pb[:, 0:span], in0=pb[:, 0:span],
                                in1=mask_sb[:, moff:moff + span], op=MULT)

                        def qk_rand(t):
                            ssc = spsum.tile([128, 512], f32, tag="s")
                            for g in range(NUM_RANDOM):
                                n0 = t * NRG + g * 128
                                nc.tensor.matmul(
                                    ssc[:, g * 128:(g + 1) * 128],
                                    kselT[hs, b, n0 // 768,
                                          n0 % 768:n0 % 768 + 128],
                                    qT[hs, base + t * 128:base + (t + 1) * 128],
                                    start=True, stop=True)
                            pr = pbpool.tile([128, 384], bf16, tag="pr",
                                             name=f"pr{b}{h}_{t}")
                            p_rand[t] = pr
                            nc.scalar.activation(pr[:], ssc[:, 0:NRG], EXP)
                            m0 = MOFF_RAND + t * NRG
                            nc.vector.tensor_tensor(
                                out=pr[:], in0=pr[:],
                                in1=mask_sb[:, m0:m0 + NRG], op=MULT)

                        def pv_glob():
                            # init whole [65, S] region (global keys attend
                            # to every query)
                            for w in range(4):
                                nc.tensor.matmul(
                                    pvo[:, w * 512:(w + 1) * 512],
                                    vaug[b][h][:, 0, 0:HD + 1],
                                    p_glob[:, w * 512:(w + 1) * 512],
                                    start=True, stop=False,
                                    skip_group_check=True)

                        def pv_band(j):
                            span = BSPAN[j]
                            ilo = TLO[j] * 128
                            # split at PSUM bank (512-col) boundaries
                            lo = ilo
                            while lo < ilo + span:
                                hi = min(ilo + span, (lo // 512 + 1) * 512)
                                nc.tensor.matmul(
                                    pvo[:, lo:hi],
                                    vaug[b][h][:, j, 0:HD + 1],
                                    p_band[j][:, lo - ilo:hi - ilo],
                                    start=False, stop=False,
                                    skip_group_check=True)
                                lo = hi
                            del p_band[j]

                        def pv_rand(t):
                            for g in range(NUM_RANDOM):
                                c = NUM_RANDOM * t + g
                                nc.tensor.matmul(
                                    pvo[:, t * 128:(t + 1) * 128],
                                    vselaug[:, c, b * HPC + h, :],
                                    p_rand[t][:, g * 128:(g + 1) * 128],
                                    start=False, stop=(g == NUM_RANDOM - 1),
                                    skip_group_check=True)
                            del p_rand[t]

                        # software-pipelined emission: QK one step ahead of PV
                        work_qk = ([qk_glob]
                                   + [lambda j=j: qk_band(j)
                                      for j in range(1, NT)]
                                   + [lambda t=t: qk_rand(t)
                                      for t in range(NT)])
                        work_pv = ([pv_glob]
                                   + [lambda j=j: pv_band(j)
                                      for j in range(1, NT)]
                                   + [lambda t=t: pv_rand(t)
                                      for t in range(NT)])
                        work_qk[0]()
                        if len(work_qk) > 1:
                            work_qk[1]()
                        for i in range(len(work_pv)):
                            if i + 2 < len(work_qk):
                                work_qk[i + 2]()
                            work_pv[i]()

                        # normalize: hout = pv / den per 512-col group
                        for w in range(4):
                            wsl = slice(w * 512, (w + 1) * 512)
                            den = norm.tile([1, 512], f32, tag="den")
                            nc.scalar.copy(den[:], pvo[HD:HD + 1, wsl])
                            bct = spsum.tile([128, 512], f32, tag="s",
                                             name="bct")
                            bc = bct[0:HD, :]
                            nc.tensor.matmul(bc[:], ones_f32[0:1, :], den[:],
                                             start=True, stop=True)
                            rbc = norm.tile([HD, 512], f32, tag="rbc")
                            nc.vector.reciprocal(rbc[:], bc[:])
                            nc.vector.tensor_tensor(
                                out=houtb[hs, b, wsl], in0=pvo[0:HD, wsl],
                                in1=rbc[:], op=MULT)

                        # global rows (i = 0, 1): dense attention, overwrite
                        NG = NUM_GLOBAL
                        q_rhs = qT[hs, base:base + NG]
                        gsc = spsum.tile([128, 512], f32, tag="s")
                        for jc in range(NT):
                            nc.tensor.matmul(
                                gsc[:, jc * NG:(jc + 1) * NG],
                                kT[hs, base + jc * 128:base + (jc + 1) * 128],
                                q_rhs, start=True, stop=True)
                        pg = norm.tile([128, NT * NG], bf16, tag="pgr")
                        nc.scalar.activation(pg[:], gsc[:, :NT * NG], EXP)
                        pvgt = spsum.tile([128, 512], f32, tag="s",
                                          name="pvgt")
                        pvg = pvgt[0:HD + 1, 0:NG]
                        for jc in range(NT):
                            nc.tensor.matmul(pvg[:],
                                             vaug[b][h][:, jc, 0:HD + 1],
                                             pg[:, jc * NG:(jc + 1) * NG],
                                             start=(jc == 0),
                                             stop=(jc == NT - 1))
                        deng = norm.tile([1, NG], f32, tag="deng")
                        nc.scalar.copy(deng[:], pvg[HD:HD + 1, :])
                        bcgt = spsum.tile([128, 512], f32, tag="s",
                                          name="bcgt")
                        bcg = bcgt[0:HD, 0:NG]
                        nc.tensor.matmul(bcg[:], ones_f32[0:1, :], deng[:],
                                         start=True, stop=True)
                        rbcg = norm.tile([HD, NG], f32, tag="rbcg")
                        nc.vector.reciprocal(rbcg[:], bcg[:])
                        nc.vector.tensor_tensor(
                            out=houtb[hs, b, 0:NG], in0=pvg[0:HD, :],
                            in1=rbcg[:], op=MULT)

        if "D" not in STAGES:
            return
        # ---- stage D: output projection, both heads contracted (K=128)
        with tc.tile_pool(name="osb", bufs=3) as opool, \
                tc.tile_pool(name="opsum", bufs=4, space="PSUM") as opsum:
            for b in range(B):
                for k in range(8):
                    ob = opool.tile([128, S], bf16, tag="ob")
                    for w in range(4):
                        po = opsum.tile([128, 512], f32, tag="po")
                        nc.tensor.matmul(
                            po[:], wo_sb[:, k * 128:(k + 1) * 128],
                            houtb[:, b, w * 512:(w + 1) * 512],
                            start=True, stop=True)
                        if w % 2 == 0:
                            nc.vector.tensor_copy(ob[:, w * 512:(w + 1) * 512],
                                                  po[:])
                        else:
                            nc.scalar.copy(ob[:, w * 512:(w + 1) * 512], po[:])
                    nc.sync.dma_start(
                        t_["outT"][k * 128:(k + 1) * 128,
                                   b * S:(b + 1) * S], ob[:])


# ---------------------------------------------------------------- execution
_NC_CACHE = None


def _get_nc():
    global _NC_CACHE
    if _NC_CACHE is None:
        _NC_CACHE = build_kernel()
    return _NC_CACHE


def _install_axon_trace_shim():
    import sys
    import types

    if "antenv.axon_hooks" in sys.modules:
        return
    mod = types.ModuleType("antenv.axon_hooks")
    mod._hook = None
    mod.set_axon_ntff_profile_hook = lambda h: setattr(mod, "_hook", h)
    mod.get_axon_ntff_profile_hook = lambda: mod._hook
    sys.modules["antenv.axon_hooks"] = mod
    try:
        import antenv
        antenv.axon_hooks = mod
        from trn_agent_boot.trn_boot import _ntff_profile_via_ctypes
        mod._hook = _ntff_profile_via_ctypes("/opt/axon/libaxon_pjrt.so")
    except Exception:
        pass


def run_on_hw(in_maps, trace=False, trace_kwargs=None):
    import os
    _install_axon_trace_shim()
    from concourse import bass_utils
    bass_utils.upload_artifacts = lambda tmpdir: f"local:{tmpdir}"
    if os.environ.get("K_LDW_OPT") and not getattr(bass_utils, "_ldw_patched", 0):
        _orig_rc = bass_utils.run_command
        def _rc(cmd, **kw):
            cmd = ["--enable-ldw-opt=true" if c == "--enable-ldw-opt=false"
                   else c for c in cmd]
            return _orig_rc(cmd, **kw)
        bass_utils.run_command = _rc
        bass_utils._ldw_patched = 1

    nc = _get_nc()
    res = bass_utils.run_bass_kernel_spmd(
        nc, in_maps, core_ids=list(range(N_CORES)), trace=trace,
        trace_kwargs=trace_kwargs or {})
    return res.results, res


def kernel(**inputs):
    in_maps = make_in_maps(inputs)
    results, _ = run_on_hw(in_maps, trace=False)
    out = np.zeros((D, R), dtype=np.float32)
    for c in range(N_CORES):
        out += results[c]["outT"]
    out = out.T + np.asarray(inputs["o_b"], dtype=np.float32)[None, :]
    return np.ascontiguousarray(out.reshape(B, S, D))


# revision 5
# speedup vs baseline: 1.1283x; 1.0294x over previous
"""BigBird sparse attention on 8 Trainium2 NeuronCores — v2.

Sharding: 16 heads across 8 cores (2 heads/core, both batches per core).
Per core: q/k/v projections for its 2 heads, block-sparse BigBird attention,
partial output projection (contracting both heads at once, K=128), output
written as outT [D, R] bf16; host sums the 8 partials (transposed) + o_b.

v2 changes vs baseline:
- Random-key K/V fetched with ROW-mode dma_gather only (no transpose-mode
  SWDGE gathers); K chunks transposed to column layout on the PE.
- V gathered with a denominator-rider column (1.0) per (b, h) so every PV
  matmul is a single M=65 accumulation (no separate denominator matmuls).
- QK/PV restructured j-major: global key block j=0 processed with N=512
  matmuls over all queries; band key blocks j>=1 with N<=384 matmuls over
  the 3 adjacent query blocks; PV accumulates into one [65, S] PSUM region
  per (b, h) using partial-region start/stop groups.
- Normalization via tensor_tensor divide against a PE-broadcast denominator.
- Output projection contracts both heads per matmul (K=128) into oT layout.
"""

import math
import numpy as np

# ---------------------------------------------------------------- constants
B = 2
S = 2048
D = 1024
H = 16
HD = 64
NUM_GLOBAL = 2
NUM_RANDOM = 3
WINDOW = 3

N_CORES = 8
HPC = H // N_CORES          # heads per core = 2
HD2 = HPC * HD              # 128 = head-dim slice per core
R = B * S                   # 4096 flattened rows
NT = S // 128               # 16 i-blocks / j-blocks
NRG = NUM_RANDOM * 128      # gathered random keys per i-block = 384
NIDX = NT * NRG             # 6144 gather indices (each row holds both b)
NCH = NIDX // 128           # 48 gathered chunks of 128 keys

INV_SQRT_HD = 1.0 / math.sqrt(float(HD))

# j-major band spans: i-block range [TLO[j], THI[j]] uses key block j as a
# band chunk. j=0 is the global pass (all i-blocks).
TLO = [0] + [max(j - 1, 0) for j in range(1, NT)]
THI = [NT - 1] + [min(j + 1, NT - 1) for j in range(1, NT)]
BSPAN = [128 * (THI[j] - TLO[j] + 1) for j in range(NT)]  # cols; j=0: 2048

# mask layout: [glob 2048][band j=1..15][rand 48*128]
MOFF_BAND = [0] * NT
off = BSPAN[0]
for j in range(1, NT):
    MOFF_BAND[j] = off
    off += BSPAN[j]
MOFF_RAND = off
NMASK = off + NCH * 128


# ---------------------------------------------------------------- host prep
def _build_ref_mask(random_indices):
    i = np.arange(S)[:, None]
    j = np.arange(S)[None, :]
    glob = (i < NUM_GLOBAL) | (j < NUM_GLOBAL)
    win = np.abs(i - j) <= WINDOW
    rand = np.zeros((S, S), dtype=bool)
    rows = np.repeat(np.arange(S), NUM_RANDOM)
    rand[rows, random_indices.reshape(-1)] = True
    return glob | win | rand


def _host_masks_and_idx(random_indices):
    """j-major band masks + rand chunk masks [128, NMASK] bf16, gather idx."""
    import ml_dtypes

    ri = np.asarray(random_indices).astype(np.int64)
    M = _build_ref_mask(ri)

    masks = np.zeros((128, NMASK), dtype=np.float32)
    # global + band: exact reference mask restricted to (j-block, i-span),
    # transposed to [j-key, i].
    for j in range(NT):
        moff = MOFF_BAND[j]
        ilo, ihi = TLO[j] * 128, (THI[j] + 1) * 128
        sub = M[ilo:ihi, j * 128:(j + 1) * 128]  # [i, j]
        masks[:, moff:moff + (ihi - ilo)] = sub.T.astype(np.float32)
    # rand chunks: c = 3t + g covers key n = 128g + p of i-block t
    for t in range(NT):
        covered = {jb for jb in (t - 1, t, t + 1) if 0 <= jb < NT} | {0}
        for g in range(NUM_RANDOM):
            c = NUM_RANDOM * t + g
            blk = np.zeros((128, 128), dtype=np.float32)
            for p in range(128):
                n = 128 * g + p
                il = n // NUM_RANDOM
                m = n % NUM_RANDOM
                i_glob = t * 128 + il
                r_ = ri[i_glob, m]
                if r_ // 128 in covered:
                    continue
                if any(ri[i_glob, mm] == r_ for mm in range(m)):
                    continue
                blk[p, il] = 1.0
            masks[:, MOFF_RAND + c * 128:MOFF_RAND + (c + 1) * 128] = blk
    masks_bf16 = masks.astype(ml_dtypes.bfloat16)

    # gather indices: flat order n = t*384 + 128*g + p
    n = np.arange(NIDX)
    t_of = n // NRG
    g_of = (n % NRG) // 128
    p_of = n % 128
    nn = 128 * g_of + p_of
    il = nn // NUM_RANDOM
    m = nn % NUM_RANDOM
    j_of = ri[t_of * 128 + il, m]
    vals = j_of.astype(np.int16)
    a16 = np.zeros((16, NIDX // 16), dtype=np.int16)
    a16[n % 16, n // 16] = vals
    return masks_bf16, np.tile(a16, (8, 1))


def make_in_maps(inputs):
    import ml_dtypes

    x = np.asarray(inputs["x"], dtype=np.float32)
    ri = np.asarray(inputs["random_indices"])
    q_w = np.asarray(inputs["q_w"], dtype=np.float32)
    k_w = np.asarray(inputs["k_w"], dtype=np.float32)
    v_w = np.asarray(inputs["v_w"], dtype=np.float32)
    o_w = np.asarray(inputs["o_w"], dtype=np.float32)
    q_b = np.asarray(inputs["q_b"], dtype=np.float32)
    k_b = np.asarray(inputs["k_b"], dtype=np.float32)
    v_b = np.asarray(inputs["v_b"], dtype=np.float32)

    xT = np.ascontiguousarray(x.reshape(R, D).T).astype(ml_dtypes.bfloat16)
    masks, gidx = _host_masks_and_idx(ri)

    q_w = q_w * INV_SQRT_HD
    q_b = q_b * INV_SQRT_HD

    in_maps = []
    for c in range(N_CORES):
        sl = slice(HD2 * c, HD2 * (c + 1))
        in_maps.append({
            "xT": xT,
            "wq": np.ascontiguousarray(q_w[sl, :].T).astype(ml_dtypes.bfloat16),
            "wk": np.ascontiguousarray(k_w[sl, :].T).astype(ml_dtypes.bfloat16),
            "wv": np.ascontiguousarray(v_w[sl, :].T).astype(ml_dtypes.bfloat16),
            "bq": np.ascontiguousarray(q_b[sl, None]),
            "bk": np.ascontiguousarray(k_b[sl, None]),
            "bv": np.ascontiguousarray(v_b[sl, None]),
            "wo": np.ascontiguousarray(o_w[:, sl].T).astype(ml_dtypes.bfloat16),
            "masks": masks,
            "gidx": gidx,
        })
    return in_maps


# ---------------------------------------------------------------- device IR
def build_kernel():
    import concourse.tile as tile
    from concourse import bacc, mybir

    nc = bacc.Bacc("TRN2", target_bir_lowering=False, debug=False,
                   num_swdge_queues=4)
    f32 = mybir.dt.float32
    bf16 = mybir.dt.bfloat16
    i16 = mybir.dt.int16

    t_ = dict(
        xT=nc.dram_tensor("xT", [D, R], bf16, kind="ExternalInput").ap(),
        wq=nc.dram_tensor("wq", [D, HD2], bf16, kind="ExternalInput").ap(),
        wk=nc.dram_tensor("wk", [D, HD2], bf16, kind="ExternalInput").ap(),
        wv=nc.dram_tensor("wv", [D, HD2], bf16, kind="ExternalInput").ap(),
        bq=nc.dram_tensor("bq", [HD2, 1], f32, kind="ExternalInput").ap(),
        bk=nc.dram_tensor("bk", [HD2, 1], f32, kind="ExternalInput").ap(),
        bv=nc.dram_tensor("bv", [HD2, 1], f32, kind="ExternalInput").ap(),
        wo=nc.dram_tensor("wo", [HD2, D], bf16, kind="ExternalInput").ap(),
        masks=nc.dram_tensor("masks", [128, NMASK], bf16,
                             kind="ExternalInput").ap(),
        gidx=nc.dram_tensor("gidx", [128, NIDX // 16], i16,
                            kind="ExternalInput").ap(),
        outT=nc.dram_tensor("outT", [D, R], bf16, kind="ExternalOutput").ap(),
        v_stage=nc.dram_tensor("v_stage", [S, B, HPC, 128], bf16).ap(),
    )

    with tile.TileContext(nc) as tc:
        _build_tc(nc, tc, t_)
    nc.compile()
    return nc


def _build_tc(nc, tc, t_):
    import os
    from contextlib import ExitStack
    STAGES = os.environ.get("K_STAGES", "ABGCD")

    import concourse.bass as bass
    from concourse import masks as cmasks, mybir

    f32 = mybir.dt.float32
    bf16 = mybir.dt.bfloat16
    EXP = mybir.ActivationFunctionType.Exp
    MULT = mybir.AluOpType.mult
    DIV = mybir.AluOpType.divide

    with ExitStack() as ctx:
        const = ctx.enter_context(tc.tile_pool(name="const", bufs=1))
        proj = ctx.enter_context(tc.tile_pool(name="proj", bufs=1))
        persist = ctx.enter_context(tc.tile_pool(name="persist", bufs=1))

        # ---- constants
        ident = const.tile([128, 128], bf16)
        cmasks.make_identity(nc, ident[:])
        ones_f32 = const.tile([128, HD], f32)
        nc.vector.memset(ones_f32[:], 1.0)

        wq_sb = const.tile([128, 8, HD2], bf16)
        wk_sb = const.tile([128, 8, HD2], bf16)
        wv_sb = const.tile([128, 8, HD2], bf16)
        for w_sb, w_d in ((wq_sb, t_["wq"]), (wk_sb, t_["wk"]),
                          (wv_sb, t_["wv"])):
            nc.sync.dma_start(w_sb[:], w_d.rearrange("(c p) m -> p c m", p=128))
        bq_sb = const.tile([HD2, 1], f32)
        bk_sb = const.tile([HD2, 1], f32)
        bv_sb = const.tile([HD2, 1], f32)
        nc.sync.dma_start(bq_sb[:], t_["bq"])
        nc.sync.dma_start(bk_sb[:], t_["bk"])
        nc.sync.dma_start(bv_sb[:], t_["bv"])
        wo_sb = const.tile([HD2, D], bf16)
        nc.sync.dma_start(wo_sb[:], t_["wo"])
        mask_sb = const.tile([128, NMASK], bf16)
        nc.sync.dma_start(mask_sb[:], t_["masks"])
        gidx_sb = const.tile([128, NIDX // 16], mybir.dt.int16)
        nc.sync.dma_start(gidx_sb[:], t_["gidx"])

        # persistent activations
        qT = proj.tile([128, R], bf16)
        kT = proj.tile([128, R], bf16)
        # both heads' attention outputs stacked: h0 -> partitions 0..63
        houtb = proj.tile([128, B, S], bf16)
        # v rows + ones rider at col HD, per (b, h)
        vaug = [[persist.tile([128, NT, 128], bf16, name=f"vaug{b}{h}")
                 for h in range(HPC)] for b in range(B)]
        for b in range(B):
            for h in range(HPC):
                nc.vector.memset(vaug[b][h][:, :, HD + 1:], 0.0)
                nc.vector.memset(vaug[b][h][:, :, HD:HD + 1], 1.0)
        # gathered tensors; vselaug free layout per chunk:
        # [b0h0 d0..63, one, pad*63, b0h1 ..., b1h0 ..., b1h1 ...]
        # kselT/vselaug are whole-tile copies made after ALL gathers land —
        # the copy's whole-tile read waits on final (order-independent) DMASW
        # sem counts, making the 4-queue gather spread race-free. vselaug is
        # compacted to the 65 used cols per (b, h).
        vselaug = persist.tile([128, NCH, B * HPC, HD + 1], bf16)
        kselT = persist.tile([128, B, NIDX // 768, 768], bf16)
        kselT0 = persist.tile([128, B, NIDX // 768, 768], bf16)
        krows_p = [persist.tile([128, NT, HD2], bf16, name=f"krowsp{b}")
                   for b in range(B)]

        # ---- stage A: projections -> qT/kT/vT [128(hd2), R] bf16
        with tc.tile_pool(name="stgA", bufs=1) as stga, \
                tc.tile_pool(name="xstream", bufs=2) as xpool, \
                tc.tile_pool(name="ppsum", bufs=3, space="PSUM") as ppsum:
            vT = stga.tile([128, R], bf16)
            xT_r = t_["xT"].rearrange("(c p) r -> p c r", p=128)

            def stage_b(b, tpsum):
                # k/v row-major staging for batch b (PE transposes), then
                # this batch's K gathers start immediately.
                krows = krows_p[b]
                for jc in range(NT):
                    csl = slice(b * S + jc * 128, b * S + (jc + 1) * 128)
                    psk = tpsum.tile([128, 128], bf16, tag="tp")
                    nc.tensor.transpose(psk[:], kT[:, csl], ident[:])
                    if jc % 2 == 0:
                        nc.vector.tensor_copy(krows[:, jc, :], psk[:])
                    else:
                        nc.scalar.copy(krows[:, jc, :], psk[:])
                    psv = tpsum.tile([128, 128], bf16, tag="tp")
                    nc.tensor.transpose(psv[:], vT[:, csl], ident[:])
                    nc.vector.tensor_copy(vaug[b][0][:, jc, 0:HD],
                                          psv[:, 0:HD])
                    nc.scalar.copy(vaug[b][1][:, jc, 0:HD],
                                   psv[:, HD:HD2])
                for h in range(HPC):
                    nc.sync.dma_start(
                        t_["v_stage"][:, b, h, :].rearrange(
                            "(c p) e -> p c e", p=128),
                        vaug[b][h][:, :, :])
                for u in range(NIDX // 768):
                    isl = slice(u * (768 // 16), (u + 1) * (768 // 16))
                    nc.gpsimd.dma_gather(
                        kselT0[:, b, u:u + 1, :], krows[:],
                        gidx_sb[:, isl], 768, 768, HD2, transpose=True,
                        queue_num=(u + 4 * b) % 4,
                        sbuf_tokens_per_rank=128,
                        sbuf_free_dim_per_rank=HD2 * 2)

            with tc.tile_pool(name="tpsumE", bufs=2, space="PSUM") as tpsumE:
                for rt in range(R // 512):
                    xt = xpool.tile([128, 8, 512], bf16)
                    nc.sync.dma_start(xt[:], xT_r[:, :, bass.ts(rt, 512)])
                    for dst, w_sb, b_sb in ((qT, wq_sb, bq_sb),
                                            (kT, wk_sb, bk_sb),
                                            (vT, wv_sb, bv_sb)):
                        ps = ppsum.tile([128, 512], f32, tag="ps")
                        for dc in range(8):
                            nc.tensor.matmul(ps[:], w_sb[:, dc, :],
                                             xt[:, dc, :],
                                             start=(dc == 0), stop=(dc == 7))
                        nc.vector.tensor_scalar_add(dst[:, bass.ts(rt, 512)],
                                                    ps[:], b_sb[:])
                    if rt == 3:
                        stage_b(0, tpsumE)
                    elif rt == 7:
                        stage_b(1, tpsumE)

        # ---- gathers: V rows from DRAM; K via SBUF-source transpose
        # gathers from krows (column-layout output, no PE transposes).
        # 768 idx per call (49 SWDGE FIFO entries), spread over 4 queues.
        CR = 768
        NCALL = NIDX // CR  # 8
        v_src = t_["v_stage"].rearrange("j b h e -> j (b h e)")
        with tc.tile_pool(name="graw", bufs=1) as graw:
            vselaug0 = graw.tile([128, NCH, B * HPC * 128], bf16)
            # queue = emission_index % 4 keeps each DMASW sem lane (index % 8)
            # on a single queue, preserving per-lane completion order.
            gq = [0]

            def nextq():
                q = gq[0] % 4
                gq[0] += 1
                return q

            nc.vector.tensor_copy(kselT[:], kselT0[:])
            for u in range(NCALL):
                isl = slice(u * (CR // 16), (u + 1) * (CR // 16))
                nc.gpsimd.dma_gather(
                    vselaug0[:, u * (CR // 128):(u + 1) * (CR // 128), :],
                    v_src, gidx_sb[:, isl], CR, CR, B * HPC * 128,
                    transpose=False, queue_num=nextq())
            v0v = vselaug0[:].rearrange("p c (x e) -> p c x e", e=128)
            nc.vector.tensor_copy(vselaug[:], v0v[:, :, :, 0:HD + 1])

            if "C" not in STAGES:
                return
            # ---- stage C: attention per (b, h)
            with tc.tile_pool(name="pglob", bufs=2) as pgpool, \
                    tc.tile_pool(name="pband", bufs=6) as pbpool, \
                    tc.tile_pool(name="norm", bufs=4) as norm, \
                    tc.tile_pool(name="spsum", bufs=3, space="PSUM") as spsum, \
                    tc.tile_pool(name="vpsum", bufs=1, space="PSUM") as vpsum:
                for b in range(B):
                    for h in range(HPC):
                        hs = slice(HD * h, HD * (h + 1))
                        base = b * S
                        pvo = vpsum.tile([HD + 1, S], f32, tag="pv")

                        # work items: QK(j) producing p, then PV(j) consuming
                        # it one step behind, to keep the PE stream dense.
                        p_glob = pgpool.tile([128, S], bf16, tag="pg")
                        p_band = {}
                        p_rand = {}

                        def qk_glob():
                            for w in range(4):
                                ssc = spsum.tile([128, 512], f32, tag="s")
                                nc.tensor.matmul(
                                    ssc[:], kT[hs, base:base + 128],
                                    qT[hs, base + w * 512:base + (w + 1) * 512],
                                    start=True, stop=True)
                                nc.scalar.activation(
                                    p_glob[:, w * 512:(w + 1) * 512], ssc[:],
                                    EXP)
                            nc.vector.tensor_tensor(
                                out=p_glob[:], in0=p_glob[:],
                                in1=mask_sb[:, 0:S], op=MULT)

                        def qk_band(j):
                            span = BSPAN[j]
                            ilo = TLO[j] * 128
                            ssc = spsum.tile([128, 512], f32, tag="s")
                            nc.tensor.matmul(
                                ssc[:, 0:span],
                                kT[hs, base + j * 128:base + (j + 1) * 128],
                                qT[hs, base + ilo:base + ilo + span],
                                start=True, stop=True)
                            pb = pbpool.tile([128, 384], bf16, tag="pb",
                                             name=f"pb{b}{h}_{j}")
                            p_band[j] = pb
                            nc.scalar.activation(pb[:, 0:span], ssc[:, 0:span],
                                                 EXP)
                            moff = MOFF_BAND[j]
                            nc.vector.tensor_tensor(
                                out=pb[:, 0:span], in0=pb[:, 0:span],
                                in1=mask_sb[:, moff:moff + span], op=MULT)

                        def qk_rand(t):
                            ssc = spsum.tile([128, 512], f32, tag="s")
                            for g in range(NUM_RANDOM):
                                n0 = t * NRG + g * 128
                                nc.tensor.matmul(
                                    ssc[:, g * 128:(g + 1) * 128],
                                    kselT[hs, b, n0 // 768,
                                          n0 % 768:n0 % 768 + 128],
                                    qT[hs, base + t * 128:base + (t + 1) * 128],
                                    start=True, stop=True)
                            pr = pbpool.tile([128, 384], bf16, tag="pr",
                                             name=f"pr{b}{h}_{t}")
                            p_rand[t] = pr
                            nc.scalar.activation(pr[:], ssc[:, 0:NRG], EXP)
                            m0 = MOFF_RAND + t * NRG
                            nc.vector.tensor_tensor(
                                out=pr[:], in0=pr[:],
                                in1=mask_sb[:, m0:m0 + NRG], op=MULT)

                        def pv_glob():
                            # init whole [65, S] region (global keys attend
                            # to every query)
                            for w in range(4):
                                nc.tensor.matmul(
                                    pvo[:, w * 512:(w + 1) * 512],
                                    vaug[b][h][:, 0, 0:HD + 1],
                                    p_glob[:, w * 512:(w + 1) * 512],
                                    start=True, stop=False,
                                    skip_group_check=True)

                        def pv_band(j):
                            span = BSPAN[j]
                            ilo = TLO[j] * 128
                            # split at PSUM bank (512-col) boundaries
                            lo = ilo
                            while lo < ilo + span:
                                hi = min(ilo + span, (lo // 512 + 1) * 512)
                                nc.tensor.matmul(
                                    pvo[:, lo:hi],
                                    vaug[b][h][:, j, 0:HD + 1],
                                    p_band[j][:, lo - ilo:hi - ilo],
                                    start=False, stop=False,
                                    skip_group_check=True)
                                lo = hi
                            del p_band[j]

                        def pv_rand(t):
                            for g in range(NUM_RANDOM):
                                c = NUM_RANDOM * t + g
                                nc.tensor.matmul(
                                    pvo[:, t * 128:(t + 1) * 128],
                                    vselaug[:, c, b * HPC + h, :],
                                    p_rand[t][:, g * 128:(g + 1) * 128],
                                    start=False, stop=(g == NUM_RANDOM - 1),
                                    skip_group_check=True)
                            del p_rand[t]

                        # software-pipelined emission: QK one step ahead of PV
                        work_qk = ([qk_glob]
                                   + [lambda j=j: qk_band(j)
                                      for j in range(1, NT)]
                                   + [lambda t=t: qk_rand(t)
                                      for t in range(NT)])
                        work_pv = ([pv_glob]
                                   + [lambda j=j: pv_band(j)
                                      for j in range(1, NT)]
                                   + [lambda t=t: pv_rand(t)
                                      for t in range(NT)])
                        work_qk[0]()
                        if len(work_qk) > 1:
                            work_qk[1]()
                        for i in range(len(work_pv)):
                            if i + 2 < len(work_qk):
                                work_qk[i + 2]()
                            work_pv[i]()

                        # normalize: hout = pv / den per 512-col group
                        for w in range(4):
                            wsl = slice(w * 512, (w + 1) * 512)
                            den = norm.tile([1, 512], f32, tag="den")
                            nc.scalar.copy(den[:], pvo[HD:HD + 1, wsl])
                            bct = spsum.tile([128, 512], f32, tag="s",
                                             name="bct")
                            bc = bct[0:HD, :]
                            nc.tensor.matmul(bc[:], ones_f32[0:1, :], den[:],
                                             start=True, stop=True)
                            rbc = norm.tile([HD, 512], f32, tag="rbc")
                            nc.vector.reciprocal(rbc[:], bc[:])
                            nc.vector.tensor_tensor(
                                out=houtb[hs, b, wsl], in0=pvo[0:HD, wsl],
                                in1=rbc[:], op=MULT)

                        # global rows (i = 0, 1): dense attention, overwrite
                        NG = NUM_GLOBAL
                        q_rhs = qT[hs, base:base + NG]
                        gsc = spsum.tile([128, 512], f32, tag="s")
                        for jc in range(NT):
                            nc.tensor.matmul(
                                gsc[:, jc * NG:(jc + 1) * NG],
                                kT[hs, base + jc * 128:base + (jc + 1) * 128],
                                q_rhs, start=True, stop=True)
                        pg = norm.tile([128, NT * NG], bf16, tag="pgr")
                        nc.scalar.activation(pg[:], gsc[:, :NT * NG], EXP)
                        pvgt = spsum.tile([128, 512], f32, tag="s",
                                          name="pvgt")
                        pvg = pvgt[0:HD + 1, 0:NG]
                        for jc in range(NT):
                            nc.tensor.matmul(pvg[:],
                                             vaug[b][h][:, jc, 0:HD + 1],
                                             pg[:, jc * NG:(jc + 1) * NG],
                                             start=(jc == 0),
                                             stop=(jc == NT - 1))
                        deng = norm.tile([1, NG], f32, tag="deng")
                        nc.scalar.copy(deng[:], pvg[HD:HD + 1, :])
                        bcgt = spsum.tile([128, 512], f32, tag="s",
                                          name="bcgt")
                        bcg = bcgt[0:HD, 0:NG]
                        nc.tensor.matmul(bcg[:], ones_f32[0:1, :], deng[:],
                                         start=True, stop=True)
                        rbcg = norm.tile([HD, NG], f32, tag="rbcg")
                        nc.vector.reciprocal(rbcg[:], bcg[:])
                        nc.vector.tensor_tensor(
                            out=houtb[hs, b, 0:NG], in0=pvg[0:HD, :],
                            in1=rbcg[:], op=MULT)

        if "D" not in STAGES:
            return
        # ---- stage D: output projection, both heads contracted (K=128)
        with tc.tile_pool(name="osb", bufs=3) as opool, \
                tc.tile_pool(name="opsum", bufs=4, space="PSUM") as opsum:
            for b in range(B):
                for k in range(8):
                    ob = opool.tile([128, S], bf16, tag="ob")
                    for w in range(4):
                        po = opsum.tile([128, 512], f32, tag="po")
                        nc.tensor.matmul(
                            po[:], wo_sb[:, k * 128:(k + 1) * 128],
                            houtb[:, b, w * 512:(w + 1) * 512],
                            start=True, stop=True)
                        if w % 2 == 0:
                            nc.vector.tensor_copy(ob[:, w * 512:(w + 1) * 512],
                                                  po[:])
                        else:
                            nc.scalar.copy(ob[:, w * 512:(w + 1) * 512], po[:])
                    nc.sync.dma_start(
                        t_["outT"][k * 128:(k + 1) * 128,
                                   b * S:(b + 1) * S], ob[:])


# ---------------------------------------------------------------- execution
_NC_CACHE = None


def _get_nc():
    global _NC_CACHE
    if _NC_CACHE is None:
        _NC_CACHE = build_kernel()
    return _NC_CACHE


def _install_axon_trace_shim():
    import sys
    import types

    if "antenv.axon_hooks" in sys.modules:
        return
    mod = types.ModuleType("antenv.axon_hooks")
    mod._hook = None
    mod.set_axon_ntff_profile_hook = lambda h: setattr(mod, "_hook", h)
    mod.get_axon_ntff_profile_hook = lambda: mod._hook
    sys.modules["antenv.axon_hooks"] = mod
    try:
        import antenv
        antenv.axon_hooks = mod
        from trn_agent_boot.trn_boot import _ntff_profile_via_ctypes
        mod._hook = _ntff_profile_via_ctypes("/opt/axon/libaxon_pjrt.so")
    except Exception:
        pass


def run_on_hw(in_maps, trace=False, trace_kwargs=None):
    import os
    _install_axon_trace_shim()
    from concourse import bass_utils
    bass_utils.upload_artifacts = lambda tmpdir: f"local:{tmpdir}"
    if os.environ.get("K_LDW_OPT") and not getattr(bass_utils, "_ldw_patched", 0):
        _orig_rc = bass_utils.run_command
        def _rc(cmd, **kw):
            cmd = ["--enable-ldw-opt=true" if c == "--enable-ldw-opt=false"
                   else c for c in cmd]
            return _orig_rc(cmd, **kw)
        bass_utils.run_command = _rc
        bass_utils._ldw_patched = 1

    nc = _get_nc()
    res = bass_utils.run_bass_kernel_spmd(
        nc, in_maps, core_ids=list(range(N_CORES)), trace=trace,
        trace_kwargs=trace_kwargs or {})
    return res.results, res


def kernel(**inputs):
    in_maps = make_in_maps(inputs)
    results, _ = run_on_hw(in_maps, trace=False)
    out = np.zeros((D, R), dtype=np.float32)
    for c in range(N_CORES):
        out += results[c]["outT"]
    out = out.T + np.asarray(inputs["o_b"], dtype=np.float32)[None, :]
    return np.ascontiguousarray(out.reshape(B, S, D))
